# revision 1
# baseline (speedup 1.0000x reference)
"""DualGraphEncoder (2-stream, 2-layer GraphSAGE-mean) on 8 Trainium2 cores.

Sharding: stream-split + node blocks, pairs adjacent.
  core k: stream g = k % 2 (0 spatial / 1 attr), dest block b = k // 2
  (12500 rows per block).
Each core aggregates the edges whose destination falls in its block via
one-hot matmul aggregation (P generated on-device from iota/is_equal with
1/deg folded in), then applies the dense SAGE layer in transposed
orientation (out^T = W^T @ x^T).

Gather layout: source nodes are split into Q=4 sub-chunks of S=3125 rows
per block; edges are bucketed by (dest tile, q). Both layers' gathers use
512B descriptors (the measured dma_gather knee: 256B descs run at ~27GB/s,
512B at ~150GB/s):
  - L0 gathers from a host-built pair table xp[q][j] = [x_j, x_{j+1}]
    (256 bf16 cols) and the aggregation matmul uses only cols 0:128.
  - L1 gathers 256-feat h0 rows (512B native).
h0 is exchanged per-q by 4 chunked AllGathers that fire as soon as the
corresponding quarter of row-major h0 is written, overlapping the L0 dense
tail and L1 gathers. The final blend w*hs + (1-w)*ha is realized by
pre-scaling layer-1 weights by sigmoid(alpha) per stream and summing the
two streams' h1 with 4 chunked bf16 AllReduces over adjacent core pairs
(2b, 2b+1), overlapped with the L1 dense.

kernel(**inputs) takes the FULL reference inputs and returns the FULL output.
"""
import sys
import os

for _p in ("/opt/trn_rl_repo", "/root/.axon_site/_ro/trn_rl_repo"):
    if os.path.isdir(_p) and _p not in sys.path:
        sys.path.insert(0, _p)

import numpy as np
import ml_dtypes

import concourse.bass as bass
import concourse.mybir as mybir
import concourse.tile as tile
import concourse.bacc as bacc

N_CORES = 8
TILE = 128
NQ = 2               # source sub-chunks per block (AllGather chunks)
T_RANGE = 4          # dest tiles per gather unit

F32 = mybir.dt.float32
BF16 = mybir.dt.bfloat16
I16 = mybir.dt.int16


class Cfg:
    def __init__(self, n, e, d_in, d_hid, d_out):
        assert n % (4 * NQ) == 0
        self.N, self.E = n, e
        self.D_IN, self.D_HID, self.D_OUT = d_in, d_hid, d_out
        self.BLOCK = n // 4          # dest rows per core block
        self.S = self.BLOCK // NQ    # source rows per (block, q) sub-chunk
        self.NT = (self.BLOCK + TILE - 1) // TILE   # dest tiles per block
        self.LAST_W = self.BLOCK - (self.NT - 1) * TILE
        self.DENSE_N = 500           # dense chunk (<=512 for one PSUM bank)
        assert self.BLOCK % self.DENSE_N == 0
        self.NJ = self.BLOCK // self.DENSE_N


FULL = Cfg(50000, 800000, 128, 256, 256)


# ---------------------------------------------------------------- host prep

def _bucket_core(row, col, inv, blk_start, cfg):
    """Bucket one core's edges by (dest tile, source q-chunk); sort each
    bucket by local source index. Returns buckets[t][q] = (cl, slot, invc)."""
    m = (row >= blk_start) & (row < blk_start + cfg.BLOCK)
    er = (row[m] - blk_start).astype(np.int64)
    ec = col[m].astype(np.int64)
    iv = inv[row[m]].astype(np.float32)
    t = er // TILE
    slot = er % TILE
    cb = ec // cfg.BLOCK
    ci = ec % cfg.BLOCK
    q = ci // cfg.S
    cl = cb * cfg.S + ci % cfg.S          # local idx into q-table [0, BLOCK)
    key = (t * NQ + q) * (4 * cfg.S) + cl  # bucket-major, then source order
    order = np.argsort(key, kind="stable")
    ks = (t * NQ + q)[order]
    bounds = np.searchsorted(ks, np.arange(cfg.NT * NQ + 1))
    buckets = [[None] * NQ for _ in range(cfg.NT)]
    for tt in range(cfg.NT):
        for qq in range(NQ):
            a, b = bounds[tt * NQ + qq], bounds[tt * NQ + qq + 1]
            sel = order[a:b]
            buckets[tt][qq] = (cl[sel], slot[sel], iv[sel])
    return buckets


def preprocess(inputs, cfg):
    x = np.asarray(inputs["x"], np.float32)
    alpha = float(np.asarray(inputs["alpha"]))
    w_blend = 1.0 / (1.0 + np.exp(-alpha))

    streams = []
    for ekey in ("edge_spatial", "edge_attr"):
        ed = np.asarray(inputs[ekey])
        row, col = ed[0].astype(np.int64), ed[1].astype(np.int64)
        cnt = np.bincount(row, minlength=cfg.N).astype(np.float64)
        inv = (1.0 / (cnt + 1e-12)).astype(np.float32)
        streams.append((row, col, inv))

    core_buckets = []
    for k in range(N_CORES):
        g, b = k % 2, k // 2
        row, col, inv = streams[g]
        core_buckets.append(_bucket_core(row, col, inv, b * cfg.BLOCK, cfg))

    # shared chunk counts (max over cores, SPMD program is identical)
    C = np.zeros((cfg.NT, NQ), np.int64)
    for t in range(cfg.NT):
        for q in range(NQ):
            mx = max(len(core_buckets[k][t][q][0]) for k in range(N_CORES))
            C[t, q] = (mx + TILE - 1) // TILE

    # schedule: edge stream order = (range, q, t); offsets in chunks
    nrange = (cfg.NT + T_RANGE - 1) // T_RANGE
    chunk_off = np.zeros((cfg.NT, NQ), np.int64)
    units = []   # (r, q, edge_off, n_edges)
    off = 0
    for r in range(nrange):
        tiles = list(range(r * T_RANGE, min((r + 1) * T_RANGE, cfg.NT)))
        for q in range(NQ):
            u0 = off
            for t in tiles:
                chunk_off[t, q] = off
                off += C[t, q]
            units.append((r, q, u0 * TILE, (off - u0) * TILE))
    totch = off
    tot = totch * TILE

    # pair table xp[q]: row j = [x_src(q,j); x_src(q,j+1)], shared by cores
    # of the same... identical for all cores (x is global).
    xbf = x.astype(ml_dtypes.bfloat16)
    trows = 4 * cfg.S
    xq_tab = np.zeros((NQ, trows + 1, cfg.D_IN), ml_dtypes.bfloat16)
    for q in range(NQ):
        src = np.concatenate(
            [xbf[bb * cfg.BLOCK + q * cfg.S:bb * cfg.BLOCK + q * cfg.S + cfg.S]
             for bb in range(4)], axis=0)          # [4*S, D_IN]
        xq_tab[q, :trows] = src
    xp = np.concatenate([xq_tab[:, :trows], xq_tab[:, 1:trows + 1]],
                        axis=2)                    # [NQ, 4*S, 2*D_IN]
    xp = np.ascontiguousarray(xp)

    in_maps = []
    for k in range(N_CORES):
        g, b = k % 2, k // 2
        buckets = core_buckets[k]
        col_l = np.zeros(tot, np.int16)
        dest_l = np.full(tot, -1.0, np.float32)
        invc_l = np.zeros(tot, np.float32)
        for t in range(cfg.NT):
            for q in range(NQ):
                cl, slot, iv = buckets[t][q]
                o = chunk_off[t, q] * TILE
                n = len(cl)
                col_l[o:o + n] = cl.astype(np.int16)
                dest_l[o:o + n] = slot.astype(np.float32)
                invc_l[o:o + n] = iv
        eidx = np.tile(col_l.reshape(tot // 16, 16).T, (8, 1))  # [128, tot/16]
        edest = dest_l.reshape(totch, TILE).T.copy()            # [128, totch]
        einvc = invc_l.reshape(totch, TILE).T.copy()

        xT = xbf[b * cfg.BLOCK:(b + 1) * cfg.BLOCK].T.copy()    # [D_IN, BLOCK]

        pre = "s" if g == 0 else "a"
        sc = np.float32(w_blend if g == 0 else 1.0 - w_blend)
        w0s = np.asarray(inputs[f"{pre}0_ws"], np.float32).astype(ml_dtypes.bfloat16)
        w0n = np.asarray(inputs[f"{pre}0_wn"], np.float32).astype(ml_dtypes.bfloat16)
        w1s = (np.asarray(inputs[f"{pre}1_ws"], np.float32) * sc).astype(ml_dtypes.bfloat16)
        w1n = (np.asarray(inputs[f"{pre}1_wn"], np.float32) * sc).astype(ml_dtypes.bfloat16)
        b0 = (np.asarray(inputs[f"{pre}0_bs"], np.float32)
              + np.asarray(inputs[f"{pre}0_bn"], np.float32))
        b1 = (np.asarray(inputs[f"{pre}1_bs"], np.float32)
              + np.asarray(inputs[f"{pre}1_bn"], np.float32)) * sc

        in_maps.append({
            "xp": xp, "xT": xT,
            "eidx": eidx, "edest": edest, "einvc": einvc,
            "w0s": w0s, "w0n": w0n,
            "w1s0": w1s[:128].copy(), "w1s1": w1s[128:].copy(),
            "w1n0": w1n[:128].copy(), "w1n1": w1n[128:].copy(),
            "b0": b0.reshape(2, 128).T.copy(),   # [128, 2]
            "b1": b1.reshape(2, 128).T.copy(),
        })

    sched = dict(C=C, chunk_off=chunk_off, units=units, totch=totch, tot=tot,
                 nrange=nrange)
    return in_maps, sched


# ---------------------------------------------------------------- program

def build_program(cfg, sched):
    NOCOLL = os.environ.get("GNN_NOCOLL") == "1"
    REPEAT = int(os.environ.get("GNN_REPEAT", "1"))
    C, chunk_off = sched["C"], sched["chunk_off"]
    totch, tot = sched["totch"], sched["tot"]
    DH = cfg.D_HID
    S = cfg.S

    nc = bacc.Bacc("TRN2", target_bir_lowering=False, debug=False,
                   num_devices=1 if NOCOLL else N_CORES,
                   dynamic_dma_scratch_size=49152)

    xp_d = nc.dram_tensor("xp", [NQ, 4 * cfg.S, 2 * cfg.D_IN], BF16,
                          kind="ExternalInput")
    xT_d = nc.dram_tensor("xT", [cfg.D_IN, cfg.BLOCK], BF16, kind="ExternalInput")
    eidx_d = nc.dram_tensor("eidx", [128, tot // 16], I16, kind="ExternalInput")
    edest_d = nc.dram_tensor("edest", [128, totch], F32, kind="ExternalInput")
    einvc_d = nc.dram_tensor("einvc", [128, totch], F32, kind="ExternalInput")
    w0s_d = nc.dram_tensor("w0s", [cfg.D_IN, DH], BF16, kind="ExternalInput")
    w0n_d = nc.dram_tensor("w0n", [cfg.D_IN, DH], BF16, kind="ExternalInput")
    w1_d = {(nm, kk): nc.dram_tensor(f"w1{nm}{kk}", [128, cfg.D_OUT], BF16,
                                     kind="ExternalInput")
            for nm in ("s", "n") for kk in (0, 1)}
    b0_d = nc.dram_tensor("b0", [128, 2], F32, kind="ExternalInput")
    b1_d = nc.dram_tensor("b1", [128, 2], F32, kind="ExternalInput")
    yT_d = nc.dram_tensor("yT", [cfg.D_OUT, cfg.BLOCK], F32, kind="ExternalOutput")
    h0full_in = ([nc.dram_tensor(f"h0full{q}", [4 * cfg.S, DH], BF16,
                                 kind="ExternalInput") for q in range(NQ)]
                 if NOCOLL else None)

    AG_GROUPS = [[0, 2, 4, 6], [1, 3, 5, 7]]
    AR_GROUPS = [[0, 1], [2, 3], [4, 5], [6, 7]]

    with tile.TileContext(nc) as tc:
        with (
            tc.tile_pool(name="const", bufs=1) as cp,
            tc.tile_pool(name="p", bufs=8) as pp,
            tc.tile_pool(name="idx", bufs=6) as ip,
            tc.tile_pool(name="stage", bufs=2) as sp,
            tc.tile_pool(name="dram", bufs=1, space="DRAM") as dram,
            tc.tile_pool(name="h0p", bufs=1) as h0p,
        ):
            # ---- constants
            edest_t = cp.tile([128, totch], F32)
            einvc_t = cp.tile([128, totch], F32)
            w0s_t = cp.tile([cfg.D_IN, DH], BF16)
            w0n_t = cp.tile([cfg.D_IN, DH], BF16)
            w1_t = {k: cp.tile([128, cfg.D_OUT], BF16, name=f"w1{k[0]}{k[1]}",
                               tag=f"w1{k[0]}{k[1]}") for k in w1_d}
            b0_t = cp.tile([128, 2], F32)
            b1_t = cp.tile([128, 2], F32)
            iota_i = cp.tile([128, TILE], I16)
            iota_bf = cp.tile([128, TILE], BF16)
            ident = cp.tile([128, TILE], BF16)
            pidx_i = cp.tile([128, 1], I16)
            pidx_f = cp.tile([128, 1], F32)

            nc.sync.dma_start(edest_t[:], edest_d[:])
            nc.sync.dma_start(einvc_t[:], einvc_d[:])
            nc.sync.dma_start(w0s_t[:], w0s_d[:])
            nc.sync.dma_start(w0n_t[:], w0n_d[:])
            for k in w1_d:
                nc.sync.dma_start(w1_t[k][:], w1_d[k][:])
            nc.sync.dma_start(b0_t[:], b0_d[:])
            nc.sync.dma_start(b1_t[:], b1_d[:])
            nc.gpsimd.iota(iota_i[:], pattern=[[1, TILE]], base=0,
                           channel_multiplier=0)
            nc.vector.tensor_copy(iota_bf[:], iota_i[:])
            nc.gpsimd.iota(pidx_i[:], pattern=[[1, 1]], base=0,
                           channel_multiplier=1)
            nc.vector.tensor_copy(pidx_f[:], pidx_i[:])
            nc.vector.tensor_scalar(ident[:], iota_bf[:], pidx_f[:], None,
                                    mybir.AluOpType.is_equal)

            # ---- DRAM bounces
            h0rm_q = [dram.tile([S, DH], BF16, name=f"h0rm{q}", tag=f"h0rm{q}")
                      for q in range(NQ)]
            h0full_q = (h0full_in if NOCOLL else
                        [dram.tile([4 * cfg.S, DH], BF16, name=f"h0f{q}",
                                   tag=f"h0f{q}") for q in range(NQ)])
            yar_in = [dram.tile([cfg.D_OUT, S], BF16, name=f"yi{c}", tag=f"yi{c}")
                      for c in range(NQ)]
            yar_out = [dram.tile([cfg.D_OUT, S], BF16, name=f"yo{c}", tag=f"yo{c}")
                       for c in range(NQ)]

            h0T = [h0p.tile([128, cfg.BLOCK], BF16, name=f"h0T{m}",
                            tag=f"h0T{m}") for m in range(2)]

            def gen_p(gc):
                p = pp.tile([128, TILE], BF16, tag="p")
                nc.vector.tensor_scalar(
                    p[:], iota_bf[:], edest_t[:, gc:gc + 1],
                    einvc_t[:, gc:gc + 1],
                    mybir.AluOpType.is_equal, mybir.AluOpType.mult)
                return p

            def gather_unit(gp, e0, ne, src_ap, tag):
                it = ip.tile([128, max(ne // 16, 1)], I16, tag="eidx")
                nc.scalar.dma_start(it[:, :ne // 16],
                                    eidx_d[:, e0 // 16:(e0 + ne) // 16])
                gt = gp.tile([128, max(ne // TILE, 1), 2 * cfg.D_IN], BF16,
                             tag=tag)
                nc.gpsimd.dma_gather(
                    gt[:, :ne // TILE, :], src_ap, it[:, :ne // 16],
                    num_idxs=ne, num_idxs_reg=ne, elem_size=2 * cfg.D_IN,
                    single_packet=False)
                return gt

            for _rep in range(REPEAT):
                # ================= L0 =================
                with nc.named_scope(f"L0_{_rep}"), \
                     tc.tile_pool(name=f"l0big{_rep}", bufs=1) as l0big, \
                     tc.tile_pool(name=f"g0p{_rep}", bufs=3) as g0p, \
                     tc.tile_pool(name=f"ps0{_rep}", bufs=2, space="PSUM") as psp:
                    neiT = l0big.tile([128, cfg.BLOCK], BF16, tag="neiT")
                    for r in range(sched["nrange"]):
                        tiles = list(range(r * T_RANGE,
                                           min((r + 1) * T_RANGE, cfg.NT)))
                        gts = {}
                        for q in range(NQ):
                            e0 = chunk_off[tiles[0], q] * TILE
                            ne = sum(C[t, q] for t in tiles) * TILE
                            if ne:
                                gts[q] = (gather_unit(g0p, e0, ne,
                                                      xp_d[q, :, :], "g0"),
                                          chunk_off[tiles[0], q])
                        for t in tiles:
                            w = TILE if t < cfg.NT - 1 else cfg.LAST_W
                            nch = int(sum(C[t, q] for q in range(NQ)))
                            if nch == 0:
                                nc.gpsimd.memset(neiT[:, t * TILE:t * TILE + w],
                                                 0.0)
                                continue
                            ps = psp.tile([128, TILE], F32, name=f"nei0_{t}",
                                          tag="nei0", bufs=4)
                            done = 0
                            for q in range(NQ):
                                if not C[t, q]:
                                    continue
                                gt, base = gts[q]
                                for c in range(int(C[t, q])):
                                    gc = int(chunk_off[t, q] + c)
                                    lc = gc - int(base)
                                    p = gen_p(gc)
                                    nc.tensor.matmul(
                                        ps[:], gt[:, lc, :cfg.D_IN], p[:],
                                        start=(done == 0),
                                        stop=(done == nch - 1))
                                    done += 1
                            nc.scalar.activation(neiT[:, t * TILE:t * TILE + w],
                                                 ps[:, :w],
                                                 mybir.ActivationFunctionType.Copy)

                    # dense L0 (xT streamed per chunk)
                    for j in range(cfg.NJ):
                        sl = slice(j * cfg.DENSE_N, (j + 1) * cfg.DENSE_N)
                        xT_j = sp.tile([cfg.D_IN, cfg.DENSE_N], BF16, tag="xTj")
                        nc.sync.dma_start(xT_j[:], xT_d[:, sl])
                        for m in range(2):
                            ps = psp.tile([128, cfg.DENSE_N], F32,
                                          name=f"d0_{m}_{j}", tag="d")
                            nc.tensor.matmul(ps[:], w0s_t[:, m * 128:(m + 1) * 128],
                                             xT_j[:], start=True, stop=False)
                            nc.tensor.matmul(ps[:], w0n_t[:, m * 128:(m + 1) * 128],
                                             neiT[:, sl], start=False, stop=True)
                            nc.scalar.activation(h0T[m][:, sl], ps[:],
                                                 mybir.ActivationFunctionType.Relu,
                                                 bias=b0_t[:, m:m + 1])

                # ======== row-major h0 (+ chunked AllGather) ========
                with nc.named_scope(f"H0X_{_rep}"), \
                     tc.tile_pool(name=f"pstr{_rep}", bufs=4, space="PSUM") as pstr:
                    for t in range(cfg.NT):
                        w = TILE if t < cfg.NT - 1 else cfg.LAST_W
                        rm = sp.tile([128, DH], BF16, tag="rm")
                        for m in range(2):
                            pst = pstr.tile([128, TILE], BF16, name=f"tr_{t}_{m}",
                                            tag="tr")
                            nc.tensor.transpose(pst[:w, :],
                                                h0T[m][:, t * TILE:t * TILE + w],
                                                ident[:])
                            if m == 0:
                                nc.vector.tensor_copy(rm[:w, :128], pst[:w, :])
                            else:
                                nc.scalar.activation(
                                    rm[:w, 128:], pst[:w, :],
                                    mybir.ActivationFunctionType.Copy)
                        # split rows across q chunk boundaries
                        r0 = t * TILE
                        for q in range(r0 // S, (r0 + w - 1) // S + 1):
                            a = max(r0, q * S)
                            bnd = min(r0 + w, (q + 1) * S)
                            nc.sync.dma_start(h0rm_q[q][a - q * S:bnd - q * S, :],
                                              rm[a - r0:bnd - r0, :])
                    if not NOCOLL:
                        for q in range(NQ):
                            nc.gpsimd.collective_compute(
                                "AllGather", mybir.AluOpType.bypass,
                                ins=[h0rm_q[q].opt()], outs=[h0full_q[q].opt()],
                                replica_groups=AG_GROUPS)

                # ================= L1 =================
                with nc.named_scope(f"L1_{_rep}"), \
                     tc.tile_pool(name=f"l1big{_rep}", bufs=1) as l1big, \
                     tc.tile_pool(name=f"g1p{_rep}", bufs=2) as g1p, \
                     tc.tile_pool(name=f"ps1{_rep}", bufs=2, space="PSUM") as psp1:
                    nei1T = [l1big.tile([128, cfg.BLOCK], BF16, name=f"nei1T{m}",
                                        tag=f"nei1T{m}") for m in range(2)]
                    # q-major gather order: units gated on AG_q don't block
                    # the Pool-engine FIFO behind later AG chunks. Partial
                    # per-q sums accumulate into nei1T (bf16) incrementally.
                    first_q = {}
                    for t in range(cfg.NT):
                        qs = [q for q in range(NQ) if C[t, q]]
                        first_q[t] = qs[0] if qs else None
                    for t in range(cfg.NT):
                        if first_q[t] is None:
                            w = TILE if t < cfg.NT - 1 else cfg.LAST_W
                            for m in range(2):
                                nc.gpsimd.memset(
                                    nei1T[m][:, t * TILE:t * TILE + w], 0.0)
                    for q in range(NQ):
                        for r in range(sched["nrange"]):
                            tiles = list(range(r * T_RANGE,
                                               min((r + 1) * T_RANGE, cfg.NT)))
                            e0 = chunk_off[tiles[0], q] * TILE
                            ne = sum(C[t, q] for t in tiles) * TILE
                            if ne == 0:
                                continue
                            gt = gather_unit(g1p, e0, ne,
                                             h0full_q[q][:, :], "g1")
                            base = chunk_off[tiles[0], q]
                            for t in tiles:
                                nq_ch = int(C[t, q])
                                if nq_ch == 0:
                                    continue
                                w = TILE if t < cfg.NT - 1 else cfg.LAST_W
                                sl = slice(t * TILE, t * TILE + w)
                                pss = [psp1.tile([128, TILE], F32,
                                                 name=f"n1_{t}_{q}_{m}",
                                                 tag="n1", bufs=6)
                                       for m in range(2)]
                                for c in range(nq_ch):
                                    gc = int(chunk_off[t, q] + c)
                                    lc = gc - int(base)
                                    p = gen_p(gc)
                                    for m in range(2):
                                        nc.tensor.matmul(
                                            pss[m][:],
                                            gt[:, lc, m * 128:(m + 1) * 128],
                                            p[:],
                                            start=(c == 0),
                                            stop=(c == nq_ch - 1))
                                for m in range(2):
                                    if q == first_q[t]:
                                        nc.vector.tensor_copy(
                                            nei1T[m][:, sl], pss[m][:, :w])
                                    else:
                                        nc.vector.scalar_tensor_tensor(
                                            nei1T[m][:, sl], pss[m][:, :w],
                                            1.0, nei1T[m][:, sl],
                                            mybir.AluOpType.mult,
                                            mybir.AluOpType.add)

                    # dense L1 -> yar chunks (bf16)
                    for j in range(cfg.NJ):
                        sl = slice(j * cfg.DENSE_N, (j + 1) * cfg.DENSE_N)
                        for m in range(2):
                            ps = psp1.tile([128, cfg.DENSE_N], F32,
                                           name=f"d1_{m}_{j}", tag="d")
                            nc.tensor.matmul(
                                ps[:], w1_t[("s", 0)][:, m * 128:(m + 1) * 128],
                                h0T[0][:, sl], start=True, stop=False)
                            nc.tensor.matmul(
                                ps[:], w1_t[("s", 1)][:, m * 128:(m + 1) * 128],
                                h0T[1][:, sl], start=False, stop=False)
                            nc.tensor.matmul(
                                ps[:], w1_t[("n", 0)][:, m * 128:(m + 1) * 128],
                                nei1T[0][:, sl], start=False, stop=False)
                            nc.tensor.matmul(
                                ps[:], w1_t[("n", 1)][:, m * 128:(m + 1) * 128],
                                nei1T[1][:, sl], start=False, stop=True)
                            st = sp.tile([128, cfg.DENSE_N], BF16, tag="h1")
                            nc.scalar.activation(st[:], ps[:],
                                                 mybir.ActivationFunctionType.Relu,
                                                 bias=b1_t[:, m:m + 1])
                            # split cols across AR chunk boundaries
                            c0 = j * cfg.DENSE_N
                            for c in range(c0 // S,
                                           (c0 + cfg.DENSE_N - 1) // S + 1):
                                a = max(c0, c * S)
                                bnd = min(c0 + cfg.DENSE_N, (c + 1) * S)
                                nc.sync.dma_start(
                                    yar_in[c][m * 128:(m + 1) * 128,
                                              a - c * S:bnd - c * S],
                                    st[:, a - c0:bnd - c0])

                # ======== chunked AllReduce + output ========
                with nc.named_scope(f"AR_{_rep}"):
                    for c in range(NQ):
                        if NOCOLL:
                            src = yar_in[c]
                        else:
                            nc.gpsimd.collective_compute(
                                "AllReduce", mybir.AluOpType.add,
                                ins=[yar_in[c].opt()], outs=[yar_out[c].opt()],
                                replica_groups=AR_GROUPS)
                            src = yar_out[c]
                        CC = 625
                        for m in range(2):
                            for cc in range(S // CC):
                                csl = slice(cc * CC, (cc + 1) * CC)
                                lt = sp.tile([128, CC], BF16, tag="lt")
                                nc.sync.dma_start(
                                    lt[:], src[m * 128:(m + 1) * 128, csl])
                                ft = sp.tile([128, CC], F32, tag="ft")
                                nc.scalar.activation(
                                    ft[:], lt[:],
                                    mybir.ActivationFunctionType.Copy)
                                nc.sync.dma_start(
                                    yT_d[m * 128:(m + 1) * 128,
                                         c * S + cc * CC:c * S + (cc + 1) * CC],
                                    ft[:])

    nc.compile()
    return nc


# ---------------------------------------------------------------- entry

_CACHE = {}


def _build(inputs, cfg):
    in_maps, sched = preprocess(inputs, cfg)
    key = (cfg.N, cfg.E, sched["tot"])
    if key not in _CACHE:
        _CACHE[key] = build_program(cfg, sched)
    return _CACHE[key], in_maps


def run_config(inputs, cfg):
    nc, in_maps = _build(inputs, cfg)
    from concourse import bass2jax
    results = bass2jax.run_bass_via_pjrt(nc, in_maps, n_cores=N_CORES)
    blocks = [results[2 * b]["yT"].T for b in range(4)]
    return np.ascontiguousarray(np.concatenate(blocks, axis=0), dtype=np.float32)


def kernel(**inputs):
    return run_config(inputs, FULL)



# revision 8
# speedup vs baseline: 1.1950x; 1.1950x over previous
"""DualGraphEncoder (2-stream, 2-layer GraphSAGE-mean) on 8 Trainium2 cores.

Sharding: stream-split + node blocks, pairs adjacent.
  core k: stream g = k % 2 (0 spatial / 1 attr), dest block b = k // 2
  (12500 rows per block).
Each core aggregates the edges whose destination falls in its block via
one-hot matmul aggregation (P generated on-device from iota/is_equal with
1/deg folded in), then applies the dense SAGE layer in transposed
orientation (out^T = W^T @ x^T).

Gather layout: source nodes are split into Q=4 sub-chunks of S=3125 rows
per block; edges are bucketed by (dest tile, q). Both layers' gathers use
512B descriptors (the measured dma_gather knee: 256B descs run at ~27GB/s,
512B at ~150GB/s):
  - L0 gathers from a host-built pair table xp[q][j] = [x_j, x_{j+1}]
    (256 bf16 cols) and the aggregation matmul uses only cols 0:128.
  - L1 gathers 256-feat h0 rows (512B native).
h0 is exchanged per-q by 4 chunked AllGathers that fire as soon as the
corresponding quarter of row-major h0 is written, overlapping the L0 dense
tail and L1 gathers. The final blend w*hs + (1-w)*ha is realized by
pre-scaling layer-1 weights by sigmoid(alpha) per stream and summing the
two streams' h1 with 4 chunked bf16 AllReduces over adjacent core pairs
(2b, 2b+1), overlapped with the L1 dense.

kernel(**inputs) takes the FULL reference inputs and returns the FULL output.
"""
import sys
import os

for _p in ("/opt/trn_rl_repo", "/root/.axon_site/_ro/trn_rl_repo"):
    if os.path.isdir(_p) and _p not in sys.path:
        sys.path.insert(0, _p)

import numpy as np
import ml_dtypes

import concourse.bass as bass
import concourse.mybir as mybir
import concourse.tile as tile
import concourse.bacc as bacc

N_CORES = 8
TILE = 128
NQ = 2               # source sub-chunks per block (AllGather chunks)
T_RANGE = int(os.environ.get("GNN_TRANGE", "4"))  # dest tiles per gather unit

F32 = mybir.dt.float32
BF16 = mybir.dt.bfloat16
I16 = mybir.dt.int16


class Cfg:
    def __init__(self, n, e, d_in, d_hid, d_out):
        assert n % (4 * NQ) == 0
        self.N, self.E = n, e
        self.D_IN, self.D_HID, self.D_OUT = d_in, d_hid, d_out
        self.BLOCK = n // 4          # dest rows per core block
        self.S = self.BLOCK // NQ    # source rows per (block, q) sub-chunk
        self.NT = (self.BLOCK + TILE - 1) // TILE   # dest tiles per block
        self.LAST_W = self.BLOCK - (self.NT - 1) * TILE
        self.DENSE_N = 500           # dense chunk (<=512 for one PSUM bank)
        assert self.BLOCK % self.DENSE_N == 0
        self.NJ = self.BLOCK // self.DENSE_N


FULL = Cfg(50000, 800000, 128, 256, 256)


# ---------------------------------------------------------------- host prep

def _bucket_core(row, col, inv, blk_start, cfg):
    """Bucket one core's edges by (dest tile, source q-chunk); sort each
    bucket by local source index. Returns buckets[t][q] = (cl, slot, invc)."""
    m = (row >= blk_start) & (row < blk_start + cfg.BLOCK)
    er = (row[m] - blk_start).astype(np.int64)
    ec = col[m].astype(np.int64)
    iv = inv[row[m]].astype(np.float32)
    t = er // TILE
    slot = er % TILE
    cb = ec // cfg.BLOCK
    ci = ec % cfg.BLOCK
    q = ci // cfg.S
    cl = cb * cfg.S + ci % cfg.S          # local idx into q-table [0, BLOCK)
    key = (t * NQ + q) * (4 * cfg.S) + cl  # bucket-major, then source order
    order = np.argsort(key, kind="stable")
    ks = (t * NQ + q)[order]
    bounds = np.searchsorted(ks, np.arange(cfg.NT * NQ + 1))
    buckets = [[None] * NQ for _ in range(cfg.NT)]
    for tt in range(cfg.NT):
        for qq in range(NQ):
            a, b = bounds[tt * NQ + qq], bounds[tt * NQ + qq + 1]
            sel = order[a:b]
            buckets[tt][qq] = (cl[sel], slot[sel], iv[sel])
    return buckets


def preprocess(inputs, cfg):
    x = np.asarray(inputs["x"], np.float32)
    alpha = float(np.asarray(inputs["alpha"]))
    w_blend = 1.0 / (1.0 + np.exp(-alpha))

    streams = []
    for ekey in ("edge_spatial", "edge_attr"):
        ed = np.asarray(inputs[ekey])
        row, col = ed[0].astype(np.int64), ed[1].astype(np.int64)
        cnt = np.bincount(row, minlength=cfg.N).astype(np.float64)
        inv = (1.0 / (cnt + 1e-12)).astype(np.float32)
        streams.append((row, col, inv))

    core_buckets = []
    for k in range(N_CORES):
        g, b = k % 2, k // 2
        row, col, inv = streams[g]
        core_buckets.append(_bucket_core(row, col, inv, b * cfg.BLOCK, cfg))

    # shared chunk counts (max over cores, SPMD program is identical)
    C = np.zeros((cfg.NT, NQ), np.int64)
    for t in range(cfg.NT):
        for q in range(NQ):
            mx = max(len(core_buckets[k][t][q][0]) for k in range(N_CORES))
            C[t, q] = (mx + TILE - 1) // TILE

    # schedule: edge stream order = (range, q, t); offsets in chunks
    nrange = (cfg.NT + T_RANGE - 1) // T_RANGE
    chunk_off = np.zeros((cfg.NT, NQ), np.int64)
    units = []   # (r, q, edge_off, n_edges)
    off = 0
    for r in range(nrange):
        tiles = list(range(r * T_RANGE, min((r + 1) * T_RANGE, cfg.NT)))
        for q in range(NQ):
            u0 = off
            for t in tiles:
                chunk_off[t, q] = off
                off += C[t, q]
            units.append((r, q, u0 * TILE, (off - u0) * TILE))
    totch = off
    tot = totch * TILE

    # pair table xp[q]: row j = [x_src(q,j); x_src(q,j+1)], shared by cores
    # of the same... identical for all cores (x is global).
    xbf = x.astype(ml_dtypes.bfloat16)
    trows = 4 * cfg.S
    xq_tab = np.zeros((NQ, trows + 1, cfg.D_IN), ml_dtypes.bfloat16)
    for q in range(NQ):
        src = np.concatenate(
            [xbf[bb * cfg.BLOCK + q * cfg.S:bb * cfg.BLOCK + q * cfg.S + cfg.S]
             for bb in range(4)], axis=0)          # [4*S, D_IN]
        xq_tab[q, :trows] = src
    xp = np.concatenate([xq_tab[:, :trows], xq_tab[:, 1:trows + 1]],
                        axis=2)                    # [NQ, 4*S, 2*D_IN]
    xp = np.ascontiguousarray(xp)

    in_maps = []
    for k in range(N_CORES):
        g, b = k % 2, k // 2
        buckets = core_buckets[k]
        col_l = np.zeros(tot, np.int16)
        dest_l = np.full(tot, -1.0, np.float32)
        invc_l = np.zeros(tot, np.float32)
        for t in range(cfg.NT):
            for q in range(NQ):
                cl, slot, iv = buckets[t][q]
                o = chunk_off[t, q] * TILE
                n = len(cl)
                col_l[o:o + n] = cl.astype(np.int16)
                dest_l[o:o + n] = slot.astype(np.float32)
                invc_l[o:o + n] = iv
        eidx = np.tile(col_l.reshape(tot // 16, 16).T, (8, 1))  # [128, tot/16]
        edest = dest_l.reshape(totch, TILE).T.copy()            # [128, totch]
        einvc = invc_l.reshape(totch, TILE).T.copy()

        xT = xbf[b * cfg.BLOCK:(b + 1) * cfg.BLOCK].T.copy()    # [D_IN, BLOCK]

        pre = "s" if g == 0 else "a"
        sc = np.float32(w_blend if g == 0 else 1.0 - w_blend)
        w0s = np.asarray(inputs[f"{pre}0_ws"], np.float32).astype(ml_dtypes.bfloat16)
        w0n = np.asarray(inputs[f"{pre}0_wn"], np.float32).astype(ml_dtypes.bfloat16)
        w1s = (np.asarray(inputs[f"{pre}1_ws"], np.float32) * sc).astype(ml_dtypes.bfloat16)
        w1n = (np.asarray(inputs[f"{pre}1_wn"], np.float32) * sc).astype(ml_dtypes.bfloat16)
        b0 = (np.asarray(inputs[f"{pre}0_bs"], np.float32)
              + np.asarray(inputs[f"{pre}0_bn"], np.float32))
        b1 = (np.asarray(inputs[f"{pre}1_bs"], np.float32)
              + np.asarray(inputs[f"{pre}1_bn"], np.float32)) * sc

        in_maps.append({
            "xp": xp, "xT": xT,
            "eidx": eidx, "edest": edest, "einvc": einvc,
            "w0s": w0s, "w0n": w0n,
            "w1s0": w1s[:128].copy(), "w1s1": w1s[128:].copy(),
            "w1n0": w1n[:128].copy(), "w1n1": w1n[128:].copy(),
            "b0": b0.reshape(2, 128).T.copy(),   # [128, 2]
            "b1": b1.reshape(2, 128).T.copy(),
        })

    sched = dict(C=C, chunk_off=chunk_off, units=units, totch=totch, tot=tot,
                 nrange=nrange)
    return in_maps, sched


# ---------------------------------------------------------------- program

def build_program(cfg, sched):
    NOCOLL = os.environ.get("GNN_NOCOLL") == "1"
    REPEAT = int(os.environ.get("GNN_REPEAT", "1"))
    NQUEUES = int(os.environ.get("GNN_QUEUES", "1"))
    SHARED_CC = os.environ.get("GNN_SHARED_CC") == "1"
    C, chunk_off = sched["C"], sched["chunk_off"]
    totch, tot = sched["totch"], sched["tot"]
    DH = cfg.D_HID
    S = cfg.S

    nc = bacc.Bacc("TRN2", target_bir_lowering=False, debug=False,
                   num_devices=1 if NOCOLL else N_CORES,
                   num_swdge_queues=NQUEUES,
                   dynamic_dma_scratch_size=49152)

    xp_d = nc.dram_tensor("xp", [NQ, 4 * cfg.S, 2 * cfg.D_IN], BF16,
                          kind="ExternalInput")
    xT_d = nc.dram_tensor("xT", [cfg.D_IN, cfg.BLOCK], BF16, kind="ExternalInput")
    eidx_d = nc.dram_tensor("eidx", [128, tot // 16], I16, kind="ExternalInput")
    edest_d = nc.dram_tensor("edest", [128, totch], F32, kind="ExternalInput")
    einvc_d = nc.dram_tensor("einvc", [128, totch], F32, kind="ExternalInput")
    w0s_d = nc.dram_tensor("w0s", [cfg.D_IN, DH], BF16, kind="ExternalInput")
    w0n_d = nc.dram_tensor("w0n", [cfg.D_IN, DH], BF16, kind="ExternalInput")
    w1_d = {(nm, kk): nc.dram_tensor(f"w1{nm}{kk}", [128, cfg.D_OUT], BF16,
                                     kind="ExternalInput")
            for nm in ("s", "n") for kk in (0, 1)}
    b0_d = nc.dram_tensor("b0", [128, 2], F32, kind="ExternalInput")
    b1_d = nc.dram_tensor("b1", [128, 2], F32, kind="ExternalInput")
    yT_d = nc.dram_tensor("yT", [cfg.D_OUT, cfg.BLOCK], F32, kind="ExternalOutput")
    h0full_in = ([nc.dram_tensor(f"h0full{q}", [4 * cfg.S, DH], BF16,
                                 kind="ExternalInput") for q in range(NQ)]
                 if NOCOLL else None)

    AG_GROUPS = [[0, 2, 4, 6], [1, 3, 5, 7]]
    AR_GROUPS = [[0, 1], [2, 3], [4, 5], [6, 7]]

    with tile.TileContext(nc) as tc:
        with (
            tc.tile_pool(name="const", bufs=1) as cp,
            tc.tile_pool(name="p", bufs=8) as pp,
            tc.tile_pool(name="idx", bufs=6) as ip,
            tc.tile_pool(name="stage", bufs=2) as sp,
            tc.tile_pool(name="dram", bufs=1, space="DRAM") as dram,
            tc.tile_pool(name="h0p", bufs=1) as h0p,
        ):
            # ---- constants
            edest_t = cp.tile([128, totch], F32)
            einvc_t = cp.tile([128, totch], F32)
            w0s_t = cp.tile([cfg.D_IN, DH], BF16)
            w0n_t = cp.tile([cfg.D_IN, DH], BF16)
            w1_t = {k: cp.tile([128, cfg.D_OUT], BF16, name=f"w1{k[0]}{k[1]}",
                               tag=f"w1{k[0]}{k[1]}") for k in w1_d}
            b0_t = cp.tile([128, 2], F32)
            b1_t = cp.tile([128, 2], F32)
            iota_i = cp.tile([128, TILE], I16)
            iota_bf = cp.tile([128, TILE], BF16)
            ident = cp.tile([128, TILE], BF16)
            pidx_i = cp.tile([128, 1], I16)
            pidx_f = cp.tile([128, 1], F32)

            nc.sync.dma_start(edest_t[:], edest_d[:])
            nc.sync.dma_start(einvc_t[:], einvc_d[:])
            nc.sync.dma_start(w0s_t[:], w0s_d[:])
            nc.sync.dma_start(w0n_t[:], w0n_d[:])
            for k in w1_d:
                nc.sync.dma_start(w1_t[k][:], w1_d[k][:])
            nc.sync.dma_start(b0_t[:], b0_d[:])
            nc.sync.dma_start(b1_t[:], b1_d[:])
            nc.gpsimd.iota(iota_i[:], pattern=[[1, TILE]], base=0,
                           channel_multiplier=0)
            nc.vector.tensor_copy(iota_bf[:], iota_i[:])
            nc.gpsimd.iota(pidx_i[:], pattern=[[1, 1]], base=0,
                           channel_multiplier=1)
            nc.vector.tensor_copy(pidx_f[:], pidx_i[:])
            nc.vector.tensor_scalar(ident[:], iota_bf[:], pidx_f[:], None,
                                    mybir.AluOpType.is_equal)

            # ---- DRAM bounces
            h0rm_q = [dram.tile([S, DH], BF16, name=f"h0rm{q}", tag=f"h0rm{q}")
                      for q in range(NQ)]
            cc_space = "Shared" if SHARED_CC else "Local"
            h0full_q = (h0full_in if NOCOLL else
                        [dram.tile([4 * cfg.S, DH], BF16, name=f"h0f{q}",
                                   tag=f"h0f{q}", addr_space=cc_space)
                         for q in range(NQ)])
            yar_in = [dram.tile([cfg.D_OUT, S], BF16, name=f"yi{c}", tag=f"yi{c}")
                      for c in range(NQ)]
            yar_out = [dram.tile([cfg.D_OUT, S], BF16, name=f"yo{c}",
                                 tag=f"yo{c}", addr_space=cc_space)
                       for c in range(NQ)]

            h0T = [h0p.tile([128, cfg.BLOCK], BF16, name=f"h0T{m}",
                            tag=f"h0T{m}") for m in range(2)]

            def gen_p(gc):
                p = pp.tile([128, TILE], BF16, tag="p")
                nc.vector.tensor_scalar(
                    p[:], iota_bf[:], edest_t[:, gc:gc + 1],
                    einvc_t[:, gc:gc + 1],
                    mybir.AluOpType.is_equal, mybir.AluOpType.mult)
                return p

            qctr = [0]

            def gather_unit(gp, e0, ne, src_ap, tag):
                it = ip.tile([128, max(ne // 16, 1)], I16, tag="eidx")
                nc.scalar.dma_start(it[:, :ne // 16],
                                    eidx_d[:, e0 // 16:(e0 + ne) // 16])
                gt = gp.tile([128, max(ne // TILE, 1), 2 * cfg.D_IN], BF16,
                             tag=tag)
                nc.gpsimd.dma_gather(
                    gt[:, :ne // TILE, :], src_ap, it[:, :ne // 16],
                    num_idxs=ne, num_idxs_reg=ne, elem_size=2 * cfg.D_IN,
                    single_packet=False, queue_num=qctr[0] % NQUEUES)
                qctr[0] += 1
                return gt

            for _rep in range(REPEAT):
                # ================= L0 =================
                with nc.named_scope(f"L0_{_rep}"), \
                     tc.tile_pool(name=f"l0big{_rep}", bufs=1) as l0big, \
                     tc.tile_pool(name=f"g0p{_rep}", bufs=max(3, NQUEUES)) as g0p, \
                     tc.tile_pool(name=f"ps0{_rep}", bufs=2, space="PSUM") as psp:
                    neiT = l0big.tile([128, cfg.BLOCK], BF16, tag="neiT")
                    for r in range(sched["nrange"]):
                        tiles = list(range(r * T_RANGE,
                                           min((r + 1) * T_RANGE, cfg.NT)))
                        gts = {}
                        for q in range(NQ):
                            e0 = chunk_off[tiles[0], q] * TILE
                            ne = sum(C[t, q] for t in tiles) * TILE
                            if ne:
                                gts[q] = (gather_unit(g0p, e0, ne,
                                                      xp_d[q, :, :], "g0"),
                                          chunk_off[tiles[0], q])
                        for t in tiles:
                            w = TILE if t < cfg.NT - 1 else cfg.LAST_W
                            nch = int(sum(C[t, q] for q in range(NQ)))
                            if nch == 0:
                                nc.gpsimd.memset(neiT[:, t * TILE:t * TILE + w],
                                                 0.0)
                                continue
                            ps = psp.tile([128, TILE], F32, name=f"nei0_{t}",
                                          tag="nei0", bufs=4)
                            done = 0
                            for q in range(NQ):
                                if not C[t, q]:
                                    continue
                                gt, base = gts[q]
                                for c in range(int(C[t, q])):
                                    gc = int(chunk_off[t, q] + c)
                                    lc = gc - int(base)
                                    p = gen_p(gc)
                                    nc.tensor.matmul(
                                        ps[:], gt[:, lc, :cfg.D_IN], p[:],
                                        start=(done == 0),
                                        stop=(done == nch - 1))
                                    done += 1
                            nc.scalar.activation(neiT[:, t * TILE:t * TILE + w],
                                                 ps[:, :w],
                                                 mybir.ActivationFunctionType.Copy)

                    # dense L0 (xT streamed per chunk)
                    for j in range(cfg.NJ):
                        sl = slice(j * cfg.DENSE_N, (j + 1) * cfg.DENSE_N)
                        xT_j = sp.tile([cfg.D_IN, cfg.DENSE_N], BF16, tag="xTj")
                        nc.sync.dma_start(xT_j[:], xT_d[:, sl])
                        for m in range(2):
                            ps = psp.tile([128, cfg.DENSE_N], F32,
                                          name=f"d0_{m}_{j}", tag="d")
                            nc.tensor.matmul(ps[:], w0s_t[:, m * 128:(m + 1) * 128],
                                             xT_j[:], start=True, stop=False)
                            nc.tensor.matmul(ps[:], w0n_t[:, m * 128:(m + 1) * 128],
                                             neiT[:, sl], start=False, stop=True)
                            nc.scalar.activation(h0T[m][:, sl], ps[:],
                                                 mybir.ActivationFunctionType.Relu,
                                                 bias=b0_t[:, m:m + 1])

                # ======== row-major h0 (+ chunked AllGather) ========
                with nc.named_scope(f"H0X_{_rep}"), \
                     tc.tile_pool(name=f"pstr{_rep}", bufs=4, space="PSUM") as pstr:
                    for t in range(cfg.NT):
                        w = TILE if t < cfg.NT - 1 else cfg.LAST_W
                        rm = sp.tile([128, DH], BF16, tag="rm")
                        for m in range(2):
                            pst = pstr.tile([128, TILE], BF16, name=f"tr_{t}_{m}",
                                            tag="tr")
                            nc.tensor.transpose(pst[:w, :],
                                                h0T[m][:, t * TILE:t * TILE + w],
                                                ident[:])
                            if m == 0:
                                nc.vector.tensor_copy(rm[:w, :128], pst[:w, :])
                            else:
                                nc.scalar.activation(
                                    rm[:w, 128:], pst[:w, :],
                                    mybir.ActivationFunctionType.Copy)
                        # split rows across q chunk boundaries
                        r0 = t * TILE
                        for q in range(r0 // S, (r0 + w - 1) // S + 1):
                            a = max(r0, q * S)
                            bnd = min(r0 + w, (q + 1) * S)
                            nc.sync.dma_start(h0rm_q[q][a - q * S:bnd - q * S, :],
                                              rm[a - r0:bnd - r0, :])
                    if not NOCOLL:
                        for q in range(NQ):
                            nc.gpsimd.collective_compute(
                                "AllGather", mybir.AluOpType.bypass,
                                ins=[h0rm_q[q].opt()], outs=[h0full_q[q].opt()],
                                replica_groups=AG_GROUPS)

                # ================= L1 =================
                with nc.named_scope(f"L1_{_rep}"), \
                     tc.tile_pool(name=f"l1big{_rep}", bufs=1) as l1big, \
                     tc.tile_pool(name=f"g1p{_rep}", bufs=max(2, NQUEUES)) as g1p, \
                     tc.tile_pool(name=f"ps1{_rep}", bufs=2, space="PSUM") as psp1:
                    nei1T = [l1big.tile([128, cfg.BLOCK], BF16, name=f"nei1T{m}",
                                        tag=f"nei1T{m}") for m in range(2)]
                    # q-major gather order: units gated on AG_q don't block
                    # the Pool-engine FIFO behind later AG chunks. Partial
                    # per-q sums accumulate into nei1T (bf16) incrementally.
                    first_q = {}
                    for t in range(cfg.NT):
                        qs = [q for q in range(NQ) if C[t, q]]
                        first_q[t] = qs[0] if qs else None
                    for t in range(cfg.NT):
                        if first_q[t] is None:
                            w = TILE if t < cfg.NT - 1 else cfg.LAST_W
                            for m in range(2):
                                nc.gpsimd.memset(
                                    nei1T[m][:, t * TILE:t * TILE + w], 0.0)
                    for q in range(NQ):
                        for r in range(sched["nrange"]):
                            tiles = list(range(r * T_RANGE,
                                               min((r + 1) * T_RANGE, cfg.NT)))
                            e0 = chunk_off[tiles[0], q] * TILE
                            ne = sum(C[t, q] for t in tiles) * TILE
                            if ne == 0:
                                continue
                            gt = gather_unit(g1p, e0, ne,
                                             h0full_q[q][:, :], "g1")
                            base = chunk_off[tiles[0], q]
                            for t in tiles:
                                nq_ch = int(C[t, q])
                                if nq_ch == 0:
                                    continue
                                w = TILE if t < cfg.NT - 1 else cfg.LAST_W
                                sl = slice(t * TILE, t * TILE + w)
                                pss = [psp1.tile([128, TILE], F32,
                                                 name=f"n1_{t}_{q}_{m}",
                                                 tag="n1", bufs=6)
                                       for m in range(2)]
                                for c in range(nq_ch):
                                    gc = int(chunk_off[t, q] + c)
                                    lc = gc - int(base)
                                    p = gen_p(gc)
                                    for m in range(2):
                                        nc.tensor.matmul(
                                            pss[m][:],
                                            gt[:, lc, m * 128:(m + 1) * 128],
                                            p[:],
                                            start=(c == 0),
                                            stop=(c == nq_ch - 1))
                                for m in range(2):
                                    if q == first_q[t]:
                                        nc.vector.tensor_copy(
                                            nei1T[m][:, sl], pss[m][:, :w])
                                    else:
                                        nc.vector.scalar_tensor_tensor(
                                            nei1T[m][:, sl], pss[m][:, :w],
                                            1.0, nei1T[m][:, sl],
                                            mybir.AluOpType.mult,
                                            mybir.AluOpType.add)

                    # dense L1 -> yar chunks (bf16)
                    for j in range(cfg.NJ):
                        sl = slice(j * cfg.DENSE_N, (j + 1) * cfg.DENSE_N)
                        for m in range(2):
                            ps = psp1.tile([128, cfg.DENSE_N], F32,
                                           name=f"d1_{m}_{j}", tag="d")
                            nc.tensor.matmul(
                                ps[:], w1_t[("s", 0)][:, m * 128:(m + 1) * 128],
                                h0T[0][:, sl], start=True, stop=False)
                            nc.tensor.matmul(
                                ps[:], w1_t[("s", 1)][:, m * 128:(m + 1) * 128],
                                h0T[1][:, sl], start=False, stop=False)
                            nc.tensor.matmul(
                                ps[:], w1_t[("n", 0)][:, m * 128:(m + 1) * 128],
                                nei1T[0][:, sl], start=False, stop=False)
                            nc.tensor.matmul(
                                ps[:], w1_t[("n", 1)][:, m * 128:(m + 1) * 128],
                                nei1T[1][:, sl], start=False, stop=True)
                            st = sp.tile([128, cfg.DENSE_N], BF16, tag="h1")
                            nc.scalar.activation(st[:], ps[:],
                                                 mybir.ActivationFunctionType.Relu,
                                                 bias=b1_t[:, m:m + 1])
                            # split cols across AR chunk boundaries
                            c0 = j * cfg.DENSE_N
                            for c in range(c0 // S,
                                           (c0 + cfg.DENSE_N - 1) // S + 1):
                                a = max(c0, c * S)
                                bnd = min(c0 + cfg.DENSE_N, (c + 1) * S)
                                nc.sync.dma_start(
                                    yar_in[c][m * 128:(m + 1) * 128,
                                              a - c * S:bnd - c * S],
                                    st[:, a - c0:bnd - c0])

                # ======== chunked AllReduce + output ========
                with nc.named_scope(f"AR_{_rep}"):
                    for c in range(NQ):
                        if NOCOLL:
                            src = yar_in[c]
                        else:
                            nc.gpsimd.collective_compute(
                                "AllReduce", mybir.AluOpType.add,
                                ins=[yar_in[c].opt()], outs=[yar_out[c].opt()],
                                replica_groups=AR_GROUPS)
                            src = yar_out[c]
                        CC = 625
                        for m in range(2):
                            for cc in range(S // CC):
                                csl = slice(cc * CC, (cc + 1) * CC)
                                lt = sp.tile([128, CC], BF16, tag="lt")
                                nc.sync.dma_start(
                                    lt[:], src[m * 128:(m + 1) * 128, csl])
                                ft = sp.tile([128, CC], F32, tag="ft")
                                nc.scalar.activation(
                                    ft[:], lt[:],
                                    mybir.ActivationFunctionType.Copy)
                                nc.sync.dma_start(
                                    yT_d[m * 128:(m + 1) * 128,
                                         c * S + cc * CC:c * S + (cc + 1) * CC],
                                    ft[:])

    nc.compile()
    return nc


# ---------------------------------------------------------------- entry

_CACHE = {}


def _build(inputs, cfg):
    in_maps, sched = preprocess(inputs, cfg)
    key = (cfg.N, cfg.E, sched["tot"])
    if key not in _CACHE:
        _CACHE[key] = build_program(cfg, sched)
    return _CACHE[key], in_maps


def run_config(inputs, cfg):
    nc, in_maps = _build(inputs, cfg)
    from concourse import bass2jax
    results = bass2jax.run_bass_via_pjrt(nc, in_maps, n_cores=N_CORES)
    blocks = [results[2 * b]["yT"].T for b in range(4)]
    return np.ascontiguousarray(np.concatenate(blocks, axis=0), dtype=np.float32)


def kernel(**inputs):
    return run_config(inputs, FULL)



# revision 26
# speedup vs baseline: 1.3608x; 1.1387x over previous
"""DualGraphEncoder (2-stream, 2-layer GraphSAGE-mean) on 8 Trainium2 cores.

Sharding: stream-split + node blocks, pairs adjacent.
  core k: stream g = k % 2 (0 spatial / 1 attr), dest block b = k // 2
  (12500 rows per block).
Each core aggregates the edges whose destination falls in its block via
one-hot matmul aggregation, then applies the dense SAGE layer in transposed
orientation (out^T = W^T @ x^T).

Gather layout: source nodes are split into Q=2 sub-chunks of S=6250 rows
per block; edges are bucketed by (dest tile, q) and padded to 128-edge
chunks. Gathers run on 4 SWDGE queues round-robin (measured: per-core
gather bandwidth is byte-bound at ~340-390 GB/s with >=2 queues, and 256B
descriptors are full-rate, so L0 gathers 256B rows of x directly and L1
gathers 512B rows of h0).

One-hot aggregation matrices are generated in one batched DVE op per
(dest-tile, q) bucket — scalar_tensor_tensor is_equal over broadcast APs —
with the 1/deg mean folded in afterwards by a per-tile multiply with a
precomputed [128, BLOCK] 1/deg table (deg==0 rows get 0).

h0 is exchanged per-q by 2 chunked AllGathers that fire as soon as the
corresponding half of row-major h0 is written, overlapping the L0 dense
tail and L1 gathers. The final blend w*hs + (1-w)*ha is realized by
pre-scaling layer-1 weights by sigmoid(alpha) per stream and summing the
two streams' h1 with 2 chunked bf16 AllReduces over adjacent core pairs
(2b, 2b+1), overlapped with the L1 dense.

kernel(**inputs) takes the FULL reference inputs and returns the FULL output.
"""
import sys
import os

for _p in ("/opt/trn_rl_repo", "/root/.axon_site/_ro/trn_rl_repo"):
    if os.path.isdir(_p) and _p not in sys.path:
        sys.path.insert(0, _p)

import numpy as np
import ml_dtypes

import concourse.bass as bass
import concourse.mybir as mybir
import concourse.tile as tile
import concourse.bacc as bacc

N_CORES = 8
TILE = 128
NQ = 2               # source sub-chunks per block (AllGather chunks)
T_RANGE = int(os.environ.get("GNN_TRANGE", "4"))  # dest tiles per gather unit

F32 = mybir.dt.float32
BF16 = mybir.dt.bfloat16
I16 = mybir.dt.int16


class Cfg:
    def __init__(self, n, e, d_in, d_hid, d_out):
        assert n % (4 * NQ) == 0
        self.N, self.E = n, e
        self.D_IN, self.D_HID, self.D_OUT = d_in, d_hid, d_out
        self.BLOCK = n // 4          # dest rows per core block
        self.S = self.BLOCK // NQ    # source rows per (block, q) sub-chunk
        self.NT = (self.BLOCK + TILE - 1) // TILE   # dest tiles per block
        self.LAST_W = self.BLOCK - (self.NT - 1) * TILE
        self.DENSE_N = 500           # dense chunk (<=512 for one PSUM bank)
        assert self.BLOCK % self.DENSE_N == 0
        self.NJ = self.BLOCK // self.DENSE_N


FULL = Cfg(50000, 800000, 128, 256, 256)


# ---------------------------------------------------------------- host prep

def _bucket_core(row, col, blk_start, cfg):
    """Bucket one core's edges by (dest tile, source q-chunk); sort each
    bucket by local source index. Returns buckets[t][q] = (cl, slot)."""
    m = (row >= blk_start) & (row < blk_start + cfg.BLOCK)
    er = (row[m] - blk_start).astype(np.int64)
    ec = col[m].astype(np.int64)
    t = er // TILE
    slot = er % TILE
    cb = ec // cfg.BLOCK
    ci = ec % cfg.BLOCK
    q = ci // cfg.S
    cl = cb * cfg.S + ci % cfg.S          # local idx into q-table [0, 4*S)
    key = (t * NQ + q) * (4 * cfg.S) + cl  # bucket-major, then source order
    order = np.argsort(key, kind="stable")
    ks = (t * NQ + q)[order]
    bounds = np.searchsorted(ks, np.arange(cfg.NT * NQ + 1))
    buckets = [[None] * NQ for _ in range(cfg.NT)]
    for tt in range(cfg.NT):
        for qq in range(NQ):
            a, b = bounds[tt * NQ + qq], bounds[tt * NQ + qq + 1]
            sel = order[a:b]
            buckets[tt][qq] = (cl[sel], slot[sel])
    return buckets


def preprocess(inputs, cfg):
    x = np.asarray(inputs["x"], np.float32)
    alpha = float(np.asarray(inputs["alpha"]))
    w_blend = 1.0 / (1.0 + np.exp(-alpha))

    streams = []
    for ekey in ("edge_spatial", "edge_attr"):
        ed = np.asarray(inputs[ekey])
        row, col = ed[0].astype(np.int64), ed[1].astype(np.int64)
        cnt = np.bincount(row, minlength=cfg.N).astype(np.float64)
        inv = np.where(cnt > 0, 1.0 / np.maximum(cnt, 1), 0.0).astype(np.float32)
        streams.append((row, col, inv))

    core_buckets = []
    for k in range(N_CORES):
        g, b = k % 2, k // 2
        row, col, _ = streams[g]
        core_buckets.append(_bucket_core(row, col, b * cfg.BLOCK, cfg))

    # shared chunk counts (max over cores, SPMD program is identical)
    C = np.zeros((cfg.NT, NQ), np.int64)
    for t in range(cfg.NT):
        for q in range(NQ):
            mx = max(len(core_buckets[k][t][q][0]) for k in range(N_CORES))
            C[t, q] = (mx + TILE - 1) // TILE

    # schedule: edge stream order = (range, q, t); offsets in chunks
    nrange = (cfg.NT + T_RANGE - 1) // T_RANGE
    chunk_off = np.zeros((cfg.NT, NQ), np.int64)
    units = []   # (r, q, edge_off, n_edges)
    off = 0
    for r in range(nrange):
        tiles = list(range(r * T_RANGE, min((r + 1) * T_RANGE, cfg.NT)))
        for q in range(NQ):
            u0 = off
            for t in tiles:
                chunk_off[t, q] = off
                off += C[t, q]
            units.append((r, q, u0 * TILE, (off - u0) * TILE))
    totch = off
    tot = totch * TILE

    # source q-tables (global x, identical on all cores): 256B bf16 rows
    xbf = x.astype(ml_dtypes.bfloat16)
    trows = 4 * cfg.S
    xq_tab = np.zeros((NQ, trows, cfg.D_IN), ml_dtypes.bfloat16)
    for q in range(NQ):
        xq_tab[q] = np.concatenate(
            [xbf[bb * cfg.BLOCK + q * cfg.S:bb * cfg.BLOCK + q * cfg.S + cfg.S]
             for bb in range(4)], axis=0)          # [4*S, D_IN]
    xq_tab = np.ascontiguousarray(xq_tab)

    in_maps = []
    for k in range(N_CORES):
        g, b = k % 2, k // 2
        buckets = core_buckets[k]
        inv_blk = streams[g][2][b * cfg.BLOCK:(b + 1) * cfg.BLOCK]
        col_l = np.zeros(tot, np.int16)
        dest_l = np.full(tot, -1.0, np.float32)
        invc_l = np.zeros(tot, np.float32)
        for t in range(cfg.NT):
            for q in range(NQ):
                cl, slot = buckets[t][q]
                o = chunk_off[t, q] * TILE
                n = len(cl)
                col_l[o:o + n] = cl.astype(np.int16)
                dest_l[o:o + n] = slot.astype(np.float32)
                invc_l[o:o + n] = inv_blk[slot + t * TILE]
        eidx = np.tile(col_l.reshape(tot // 16, 16).T, (8, 1))  # [128, tot/16]
        edest = np.ascontiguousarray(dest_l.reshape(totch, TILE).T.astype(
            ml_dtypes.bfloat16))                                # [128, totch]
        einvc = np.ascontiguousarray(invc_l.reshape(totch, TILE).T.astype(
            ml_dtypes.bfloat16))                                # [128, totch]

        xT = xbf[b * cfg.BLOCK:(b + 1) * cfg.BLOCK].T.copy()    # [D_IN, BLOCK]

        pre = "s" if g == 0 else "a"
        sc = np.float32(w_blend if g == 0 else 1.0 - w_blend)
        w0s = np.asarray(inputs[f"{pre}0_ws"], np.float32).astype(ml_dtypes.bfloat16)
        w0n = np.asarray(inputs[f"{pre}0_wn"], np.float32).astype(ml_dtypes.bfloat16)
        w1s = (np.asarray(inputs[f"{pre}1_ws"], np.float32) * sc).astype(ml_dtypes.bfloat16)
        w1n = (np.asarray(inputs[f"{pre}1_wn"], np.float32) * sc).astype(ml_dtypes.bfloat16)
        b0 = (np.asarray(inputs[f"{pre}0_bs"], np.float32)
              + np.asarray(inputs[f"{pre}0_bn"], np.float32))
        b1 = (np.asarray(inputs[f"{pre}1_bs"], np.float32)
              + np.asarray(inputs[f"{pre}1_bn"], np.float32)) * sc

        in_maps.append({
            "xp": xq_tab, "xT": xT,
            "eidx": eidx, "edest": edest, "einvc": einvc,
            "w0s": w0s, "w0n": w0n,
            "w1s0": w1s[:128].copy(), "w1s1": w1s[128:].copy(),
            "w1n0": w1n[:128].copy(), "w1n1": w1n[128:].copy(),
            "b0": b0.reshape(2, 128).T.copy(),   # [128, 2]
            "b1": b1.reshape(2, 128).T.copy(),
        })

    sched = dict(C=C, chunk_off=chunk_off, units=units, totch=totch, tot=tot,
                 nrange=nrange, maxch=int(C.max()))
    return in_maps, sched


# ---------------------------------------------------------------- program

def build_program(cfg, sched):
    NOCOLL = os.environ.get("GNN_NOCOLL") == "1"
    REPEAT = int(os.environ.get("GNN_REPEAT", "1"))
    NQUEUES = int(os.environ.get("GNN_QUEUES", "4"))
    SHARED_CC = os.environ.get("GNN_SHARED_CC") == "1"
    SKIPAGG = os.environ.get("GNN_SKIPAGG") == "1"   # timing ablation only
    G0BUFS = int(os.environ.get("GNN_G0BUFS", "6"))
    G1BUFS = int(os.environ.get("GNN_G1BUFS", "3"))
    C, chunk_off = sched["C"], sched["chunk_off"]
    totch, tot = sched["totch"], sched["tot"]
    MAXCH = sched["maxch"]
    DH = cfg.D_HID
    S = cfg.S

    nc = bacc.Bacc("TRN2", target_bir_lowering=False, debug=False,
                   num_devices=1 if NOCOLL else N_CORES,
                   num_swdge_queues=NQUEUES,
                   dynamic_dma_scratch_size=int(
                       os.environ.get("GNN_SCRATCH", "36864")))

    xp_d = nc.dram_tensor("xp", [NQ, 4 * cfg.S, cfg.D_IN], BF16,
                          kind="ExternalInput")
    xT_d = nc.dram_tensor("xT", [cfg.D_IN, cfg.BLOCK], BF16, kind="ExternalInput")
    eidx_d = nc.dram_tensor("eidx", [128, tot // 16], I16, kind="ExternalInput")
    edest_d = nc.dram_tensor("edest", [128, totch], BF16, kind="ExternalInput")
    einvc_d = nc.dram_tensor("einvc", [128, totch], BF16, kind="ExternalInput")
    w0s_d = nc.dram_tensor("w0s", [cfg.D_IN, DH], BF16, kind="ExternalInput")
    w0n_d = nc.dram_tensor("w0n", [cfg.D_IN, DH], BF16, kind="ExternalInput")
    w1_d = {(nm, kk): nc.dram_tensor(f"w1{nm}{kk}", [128, cfg.D_OUT], BF16,
                                     kind="ExternalInput")
            for nm in ("s", "n") for kk in (0, 1)}
    b0_d = nc.dram_tensor("b0", [128, 2], F32, kind="ExternalInput")
    b1_d = nc.dram_tensor("b1", [128, 2], F32, kind="ExternalInput")
    yT_d = nc.dram_tensor("yT", [cfg.D_OUT, cfg.BLOCK], F32, kind="ExternalOutput")
    h0full_in = ([nc.dram_tensor(f"h0full{q}", [4 * cfg.S, DH], BF16,
                                 kind="ExternalInput") for q in range(NQ)]
                 if NOCOLL else None)

    AG_GROUPS = [[0, 2, 4, 6], [1, 3, 5, 7]]
    AR_GROUPS = [[0, 1], [2, 3], [4, 5], [6, 7]]

    with tile.TileContext(nc) as tc:
        with (
            tc.tile_pool(name="const", bufs=1) as cp,
            tc.tile_pool(name="p", bufs=4) as pp,
            tc.tile_pool(name="idx", bufs=6) as ip,
            tc.tile_pool(name="stage", bufs=2) as sp,
            tc.tile_pool(name="dram", bufs=1, space="DRAM") as dram,
            tc.tile_pool(name="h0p", bufs=1) as h0p,
        ):
            # ---- constants
            edest_t = cp.tile([128, totch], BF16)
            einvc_t = cp.tile([128, totch], BF16)
            w0s_t = cp.tile([cfg.D_IN, DH], BF16)
            w0n_t = cp.tile([cfg.D_IN, DH], BF16)
            w1_t = {k: cp.tile([128, cfg.D_OUT], BF16, name=f"w1{k[0]}{k[1]}",
                               tag=f"w1{k[0]}{k[1]}") for k in w1_d}
            b0_t = cp.tile([128, 2], F32)
            b1_t = cp.tile([128, 2], F32)
            iota_i = cp.tile([128, TILE], I16)
            iota_bf = cp.tile([128, TILE], BF16)
            ident = cp.tile([128, TILE], BF16)
            pidx_i = cp.tile([128, 1], I16)
            pidx_f = cp.tile([128, 1], F32)

            nc.sync.dma_start(edest_t[:], edest_d[:])
            nc.sync.dma_start(einvc_t[:], einvc_d[:])
            nc.sync.dma_start(w0s_t[:], w0s_d[:])
            nc.sync.dma_start(w0n_t[:], w0n_d[:])
            for k in w1_d:
                nc.sync.dma_start(w1_t[k][:], w1_d[k][:])
            nc.sync.dma_start(b0_t[:], b0_d[:])
            nc.sync.dma_start(b1_t[:], b1_d[:])
            nc.gpsimd.iota(iota_i[:], pattern=[[1, TILE]], base=0,
                           channel_multiplier=0)
            nc.vector.tensor_copy(iota_bf[:], iota_i[:])
            nc.gpsimd.iota(pidx_i[:], pattern=[[1, 1]], base=0,
                           channel_multiplier=1)
            nc.vector.tensor_copy(pidx_f[:], pidx_i[:])
            nc.vector.tensor_scalar(ident[:], iota_bf[:], pidx_f[:], None,
                                    mybir.AluOpType.is_equal)

            # ---- DRAM bounces
            h0rm_q = [dram.tile([S, DH], BF16, name=f"h0rm{q}", tag=f"h0rm{q}")
                      for q in range(NQ)]
            cc_space = "Shared" if SHARED_CC else "Local"
            h0full_q = (h0full_in if NOCOLL else
                        [dram.tile([4 * cfg.S, DH], BF16, name=f"h0f{q}",
                                   tag=f"h0f{q}", addr_space=cc_space)
                         for q in range(NQ)])
            yar_in = [dram.tile([cfg.D_OUT, S], BF16, name=f"yi{c}", tag=f"yi{c}")
                      for c in range(NQ)]
            yar_out = [dram.tile([cfg.D_OUT, S], BF16, name=f"yo{c}",
                                 tag=f"yo{c}", addr_space=cc_space)
                       for c in range(NQ)]

            h0T = [h0p.tile([128, cfg.BLOCK], BF16, name=f"h0T{m}",
                            tag=f"h0T{m}") for m in range(2)]

            def gen_pb(gc0, nch):
                """One-hot matrices for nch chunks starting at gc0, one fused
                is_equal+mult DVE pass per chunk (element-bound either way;
                fusing the 1/deg mult halves the element count vs two passes).
                pb[:, c, j] = (iota[j] == edest[:, gc0+c]) * einvc[:, gc0+c]"""
                pb = pp.tile([128, MAXCH, TILE], BF16, tag="p")
                for c in range(nch):
                    gc = gc0 + c
                    nc.vector.tensor_scalar(
                        pb[:, c, :], iota_bf[:], edest_t[:, gc:gc + 1],
                        einvc_t[:, gc:gc + 1],
                        mybir.AluOpType.is_equal, mybir.AluOpType.mult)
                return pb

            qctr = [0]

            def gather_unit(gp, e0, ne, src_ap, tag, elem):
                it = ip.tile([128, max(ne // 16, 1)], I16, tag="eidx")
                nc.scalar.dma_start(it[:, :ne // 16],
                                    eidx_d[:, e0 // 16:(e0 + ne) // 16])
                gt = gp.tile([128, max(ne // TILE, 1), elem], BF16, tag=tag)
                nc.gpsimd.dma_gather(
                    gt[:, :ne // TILE, :], src_ap, it[:, :ne // 16],
                    num_idxs=ne, num_idxs_reg=ne, elem_size=elem,
                    single_packet=False, queue_num=qctr[0] % NQUEUES)
                qctr[0] += 1
                return gt

            for _rep in range(REPEAT):
                # ================= L0 =================
                with nc.named_scope(f"L0_{_rep}"), \
                     tc.tile_pool(name=f"l0big{_rep}", bufs=1) as l0big, \
                     tc.tile_pool(name=f"g0p{_rep}", bufs=G0BUFS) as g0p, \
                     tc.tile_pool(name=f"ps0{_rep}", bufs=2, space="PSUM") as psp:
                    neiT = l0big.tile([128, cfg.BLOCK], BF16, tag="neiT")
                    if SKIPAGG:
                        nc.vector.memset(neiT[:], 0.0)
                    for r in range(sched["nrange"] if not SKIPAGG else 0):
                        tiles = list(range(r * T_RANGE,
                                           min((r + 1) * T_RANGE, cfg.NT)))
                        gts = {}
                        for q in range(NQ):
                            e0 = chunk_off[tiles[0], q] * TILE
                            ne = sum(C[t, q] for t in tiles) * TILE
                            if ne:
                                gts[q] = (gather_unit(g0p, e0, ne,
                                                      xp_d[q, :, :], "g0",
                                                      cfg.D_IN),
                                          chunk_off[tiles[0], q])
                        for t in tiles:
                            w = TILE if t < cfg.NT - 1 else cfg.LAST_W
                            nch = int(sum(C[t, q] for q in range(NQ)))
                            if nch == 0:
                                nc.vector.memset(neiT[:, t * TILE:t * TILE + w],
                                                 0.0)
                                continue
                            ps = psp.tile([128, TILE], F32, name=f"nei0_{t}",
                                          tag="nei0", bufs=4)
                            done = 0
                            for q in range(NQ):
                                nq_ch = int(C[t, q])
                                if not nq_ch:
                                    continue
                                gt, base = gts[q]
                                gc0 = int(chunk_off[t, q])
                                pb = gen_pb(gc0, nq_ch)
                                for c in range(nq_ch):
                                    lc = gc0 + c - int(base)
                                    nc.tensor.matmul(
                                        ps[:], gt[:, lc, :], pb[:, c, :],
                                        start=(done == 0),
                                        stop=(done == nch - 1))
                                    done += 1
                            nc.scalar.activation(neiT[:, t * TILE:t * TILE + w],
                                                 ps[:, :w],
                                                 mybir.ActivationFunctionType.Copy)

                    # dense L0 (xT streamed per chunk)
                    for j in range(cfg.NJ):
                        sl = slice(j * cfg.DENSE_N, (j + 1) * cfg.DENSE_N)
                        xT_j = sp.tile([cfg.D_IN, cfg.DENSE_N], BF16, tag="xTj")
                        nc.sync.dma_start(xT_j[:], xT_d[:, sl])
                        for m in range(2):
                            ps = psp.tile([128, cfg.DENSE_N], F32,
                                          name=f"d0_{m}_{j}", tag="d")
                            nc.tensor.matmul(ps[:], w0s_t[:, m * 128:(m + 1) * 128],
                                             xT_j[:], start=True, stop=False)
                            nc.tensor.matmul(ps[:], w0n_t[:, m * 128:(m + 1) * 128],
                                             neiT[:, sl], start=False, stop=True)
                            nc.scalar.activation(h0T[m][:, sl], ps[:],
                                                 mybir.ActivationFunctionType.Relu,
                                                 bias=b0_t[:, m:m + 1])

                # ======== row-major h0 (+ chunked AllGather) ========
                with nc.named_scope(f"H0X_{_rep}"), \
                     tc.tile_pool(name=f"pstr{_rep}", bufs=4, space="PSUM") as pstr:
                    for t in range(cfg.NT):
                        w = TILE if t < cfg.NT - 1 else cfg.LAST_W
                        rm = sp.tile([128, DH], BF16, tag="rm")
                        for m in range(2):
                            pst = pstr.tile([128, TILE], BF16, name=f"tr_{t}_{m}",
                                            tag="tr")
                            nc.tensor.transpose(pst[:w, :],
                                                h0T[m][:, t * TILE:t * TILE + w],
                                                ident[:])
                            if m == 0:
                                nc.vector.tensor_copy(rm[:w, :128], pst[:w, :])
                            else:
                                nc.scalar.activation(
                                    rm[:w, 128:], pst[:w, :],
                                    mybir.ActivationFunctionType.Copy)
                        # split rows across q chunk boundaries
                        r0 = t * TILE
                        for q in range(r0 // S, (r0 + w - 1) // S + 1):
                            a = max(r0, q * S)
                            bnd = min(r0 + w, (q + 1) * S)
                            nc.sync.dma_start(h0rm_q[q][a - q * S:bnd - q * S, :],
                                              rm[a - r0:bnd - r0, :])
                    if not NOCOLL:
                        for q in range(NQ):
                            nc.gpsimd.collective_compute(
                                "AllGather", mybir.AluOpType.bypass,
                                ins=[h0rm_q[q].opt()], outs=[h0full_q[q].opt()],
                                replica_groups=AG_GROUPS)

                # ================= L1 =================
                with nc.named_scope(f"L1_{_rep}"), \
                     tc.tile_pool(name=f"l1big{_rep}", bufs=1) as l1big, \
                     tc.tile_pool(name=f"g1p{_rep}", bufs=G1BUFS) as g1p, \
                     tc.tile_pool(name=f"ps1{_rep}", bufs=2, space="PSUM") as psp1:
                    nei1T = [l1big.tile([128, cfg.BLOCK], BF16, name=f"nei1T{m}",
                                        tag=f"nei1T{m}") for m in range(2)]
                    # q-major gather order: units gated on AG_q don't block
                    # the Pool-engine FIFO behind later AG chunks. Partial
                    # per-q sums accumulate into nei1T (bf16) incrementally.
                    first_q = {}
                    for t in range(cfg.NT):
                        qs = [q for q in range(NQ) if C[t, q]]
                        first_q[t] = qs[0] if qs else None
                    for t in range(cfg.NT):
                        if first_q[t] is None:
                            w = TILE if t < cfg.NT - 1 else cfg.LAST_W
                            for m in range(2):
                                nc.vector.memset(
                                    nei1T[m][:, t * TILE:t * TILE + w], 0.0)
                    if SKIPAGG:
                        for m in range(2):
                            nc.vector.memset(nei1T[m][:], 0.0)
                    for q in range(NQ if not SKIPAGG else 0):
                        for r in range(sched["nrange"]):
                            tiles = list(range(r * T_RANGE,
                                               min((r + 1) * T_RANGE, cfg.NT)))
                            e0 = chunk_off[tiles[0], q] * TILE
                            ne = sum(C[t, q] for t in tiles) * TILE
                            if ne == 0:
                                continue
                            gt = gather_unit(g1p, e0, ne, h0full_q[q][:, :],
                                             "g1", 2 * cfg.D_IN)
                            base = chunk_off[tiles[0], q]
                            for t in tiles:
                                nq_ch = int(C[t, q])
                                if nq_ch == 0:
                                    continue
                                w = TILE if t < cfg.NT - 1 else cfg.LAST_W
                                sl = slice(t * TILE, t * TILE + w)
                                pss = [psp1.tile([128, TILE], F32,
                                                 name=f"n1_{t}_{q}_{m}",
                                                 tag="n1", bufs=6)
                                       for m in range(2)]
                                gc0 = int(chunk_off[t, q])
                                pb = gen_pb(gc0, nq_ch)
                                for c in range(nq_ch):
                                    lc = gc0 + c - int(base)
                                    for m in range(2):
                                        nc.tensor.matmul(
                                            pss[m][:],
                                            gt[:, lc, m * 128:(m + 1) * 128],
                                            pb[:, c, :],
                                            start=(c == 0),
                                            stop=(c == nq_ch - 1))
                                for m in range(2):
                                    if q == first_q[t]:
                                        nc.vector.tensor_copy(
                                            nei1T[m][:, sl], pss[m][:, :w])
                                    else:
                                        nc.vector.scalar_tensor_tensor(
                                            nei1T[m][:, sl], pss[m][:, :w],
                                            1.0, nei1T[m][:, sl],
                                            mybir.AluOpType.mult,
                                            mybir.AluOpType.add)

                    # dense L1 -> yar chunks (bf16)
                    for j in range(cfg.NJ):
                        sl = slice(j * cfg.DENSE_N, (j + 1) * cfg.DENSE_N)
                        for m in range(2):
                            ps = psp1.tile([128, cfg.DENSE_N], F32,
                                           name=f"d1_{m}_{j}", tag="d")
                            nc.tensor.matmul(
                                ps[:], w1_t[("s", 0)][:, m * 128:(m + 1) * 128],
                                h0T[0][:, sl], start=True, stop=False)
                            nc.tensor.matmul(
                                ps[:], w1_t[("s", 1)][:, m * 128:(m + 1) * 128],
                                h0T[1][:, sl], start=False, stop=False)
                            nc.tensor.matmul(
                                ps[:], w1_t[("n", 0)][:, m * 128:(m + 1) * 128],
                                nei1T[0][:, sl], start=False, stop=False)
                            nc.tensor.matmul(
                                ps[:], w1_t[("n", 1)][:, m * 128:(m + 1) * 128],
                                nei1T[1][:, sl], start=False, stop=True)
                            st = sp.tile([128, cfg.DENSE_N], BF16, tag="h1")
                            nc.scalar.activation(st[:], ps[:],
                                                 mybir.ActivationFunctionType.Relu,
                                                 bias=b1_t[:, m:m + 1])
                            # split cols across AR chunk boundaries
                            c0 = j * cfg.DENSE_N
                            for c in range(c0 // S,
                                           (c0 + cfg.DENSE_N - 1) // S + 1):
                                a = max(c0, c * S)
                                bnd = min(c0 + cfg.DENSE_N, (c + 1) * S)
                                nc.sync.dma_start(
                                    yar_in[c][m * 128:(m + 1) * 128,
                                              a - c * S:bnd - c * S],
                                    st[:, a - c0:bnd - c0])

                # ======== chunked AllReduce + output ========
                with nc.named_scope(f"AR_{_rep}"):
                    for c in range(NQ):
                        if NOCOLL:
                            src = yar_in[c]
                        else:
                            nc.gpsimd.collective_compute(
                                "AllReduce", mybir.AluOpType.add,
                                ins=[yar_in[c].opt()], outs=[yar_out[c].opt()],
                                replica_groups=AR_GROUPS)
                            src = yar_out[c]
                        CC = 625
                        for m in range(2):
                            for cc in range(S // CC):
                                csl = slice(cc * CC, (cc + 1) * CC)
                                lt = sp.tile([128, CC], BF16, tag="lt")
                                nc.sync.dma_start(
                                    lt[:], src[m * 128:(m + 1) * 128, csl])
                                ft = sp.tile([128, CC], F32, tag="ft")
                                nc.scalar.activation(
                                    ft[:], lt[:],
                                    mybir.ActivationFunctionType.Copy)
                                nc.sync.dma_start(
                                    yT_d[m * 128:(m + 1) * 128,
                                         c * S + cc * CC:c * S + (cc + 1) * CC],
                                    ft[:])

    nc.compile()
    return nc


# ---------------------------------------------------------------- entry

_CACHE = {}


def _build(inputs, cfg):
    in_maps, sched = preprocess(inputs, cfg)
    key = (cfg.N, cfg.E, sched["tot"])
    if key not in _CACHE:
        _CACHE[key] = build_program(cfg, sched)
    return _CACHE[key], in_maps


def run_config(inputs, cfg):
    nc, in_maps = _build(inputs, cfg)
    from concourse import bass2jax
    results = bass2jax.run_bass_via_pjrt(nc, in_maps, n_cores=N_CORES)
    blocks = [results[2 * b]["yT"].T for b in range(4)]
    return np.ascontiguousarray(np.concatenate(blocks, axis=0), dtype=np.float32)


def kernel(**inputs):
    return run_config(inputs, FULL)


# revision 39
# speedup vs baseline: 2.1587x; 1.5863x over previous
"""DualGraphEncoder (2-stream, 2-layer GraphSAGE-mean) on 8 Trainium2 cores.

Sharding: stream-split + node blocks, pairs adjacent.
  core k: stream g = k % 2 (0 spatial / 1 attr), dest block b = k // 2
  (12500 rows per block).
Each core aggregates the edges whose destination falls in its block via
one-hot matmul aggregation, then applies the dense SAGE layer in transposed
orientation (out^T = W^T @ x^T).

Gather layout: source nodes are split into Q=2 sub-chunks of S=6250 rows
per block; edges are bucketed by (dest tile, q) and padded to 128-edge
chunks. Gathers run on 4 SWDGE queues round-robin (measured: per-core
gather bandwidth is byte-bound at ~340-390 GB/s with >=2 queues, and 256B
descriptors are full-rate, so L0 gathers 256B rows of x directly and L1
gathers 512B rows of h0).

One-hot aggregation matrices are generated in one batched DVE op per
(dest-tile, q) bucket — scalar_tensor_tensor is_equal over broadcast APs —
with the 1/deg mean folded in afterwards by a per-tile multiply with a
precomputed [128, BLOCK] 1/deg table (deg==0 rows get 0).

h0 is exchanged per-q by 2 chunked AllGathers that fire as soon as the
corresponding half of row-major h0 is written, overlapping the L0 dense
tail and L1 gathers. The final blend w*hs + (1-w)*ha is realized by
pre-scaling layer-1 weights by sigmoid(alpha) per stream and summing the
two streams' h1 with 2 chunked bf16 AllReduces over adjacent core pairs
(2b, 2b+1), overlapped with the L1 dense.

kernel(**inputs) takes the FULL reference inputs and returns the FULL output.
"""
import sys
import os

for _p in ("/opt/trn_rl_repo", "/root/.axon_site/_ro/trn_rl_repo"):
    if os.path.isdir(_p) and _p not in sys.path:
        sys.path.insert(0, _p)

import numpy as np
import ml_dtypes

import concourse.bass as bass
import concourse.mybir as mybir
import concourse.tile as tile
import concourse.bacc as bacc

N_CORES = 8
TILE = 128
NQ = 2               # source sub-chunks per block (AllGather chunks)
T_RANGE = int(os.environ.get("GNN_TRANGE", "4"))  # dest tiles per gather unit

F32 = mybir.dt.float32
BF16 = mybir.dt.bfloat16
I16 = mybir.dt.int16


class Cfg:
    def __init__(self, n, e, d_in, d_hid, d_out):
        assert n % (4 * NQ) == 0
        self.N, self.E = n, e
        self.D_IN, self.D_HID, self.D_OUT = d_in, d_hid, d_out
        self.BLOCK = n // 4          # dest rows per core block
        self.S = self.BLOCK // NQ    # source rows per (block, q) sub-chunk
        self.NT = (self.BLOCK + TILE - 1) // TILE   # dest tiles per block
        self.LAST_W = self.BLOCK - (self.NT - 1) * TILE
        self.DENSE_N = 500           # dense chunk (<=512 for one PSUM bank)
        assert self.BLOCK % self.DENSE_N == 0
        self.NJ = self.BLOCK // self.DENSE_N


FULL = Cfg(50000, 800000, 128, 256, 256)


# ---------------------------------------------------------------- host prep

def _bucket_core(row, col, blk_start, cfg):
    """Bucket one core's edges by (dest tile, source q-chunk); sort each
    bucket by local source index. Returns buckets[t][q] = (cl, slot)."""
    m = (row >= blk_start) & (row < blk_start + cfg.BLOCK)
    er = (row[m] - blk_start).astype(np.int64)
    ec = col[m].astype(np.int64)
    t = er // TILE
    slot = er % TILE
    cb = ec // cfg.BLOCK
    ci = ec % cfg.BLOCK
    q = ci // cfg.S
    cl = cb * cfg.S + ci % cfg.S          # local idx into q-table [0, 4*S)
    key = (t * NQ + q) * (4 * cfg.S) + cl  # bucket-major, then source order
    order = np.argsort(key, kind="stable")
    ks = (t * NQ + q)[order]
    bounds = np.searchsorted(ks, np.arange(cfg.NT * NQ + 1))
    buckets = [[None] * NQ for _ in range(cfg.NT)]
    for tt in range(cfg.NT):
        for qq in range(NQ):
            a, b = bounds[tt * NQ + qq], bounds[tt * NQ + qq + 1]
            sel = order[a:b]
            buckets[tt][qq] = (cl[sel], slot[sel])
    return buckets


def preprocess(inputs, cfg):
    x = np.asarray(inputs["x"], np.float32)
    alpha = float(np.asarray(inputs["alpha"]))
    w_blend = 1.0 / (1.0 + np.exp(-alpha))

    streams = []
    for ekey in ("edge_spatial", "edge_attr"):
        ed = np.asarray(inputs[ekey])
        row, col = ed[0].astype(np.int64), ed[1].astype(np.int64)
        cnt = np.bincount(row, minlength=cfg.N).astype(np.float64)
        inv = np.where(cnt > 0, 1.0 / np.maximum(cnt, 1), 0.0).astype(np.float32)
        streams.append((row, col, inv))

    core_buckets = []
    for k in range(N_CORES):
        g, b = k % 2, k // 2
        row, col, _ = streams[g]
        core_buckets.append(_bucket_core(row, col, b * cfg.BLOCK, cfg))

    # shared chunk counts (max over cores, SPMD program is identical)
    C = np.zeros((cfg.NT, NQ), np.int64)
    for t in range(cfg.NT):
        for q in range(NQ):
            mx = max(len(core_buckets[k][t][q][0]) for k in range(N_CORES))
            C[t, q] = (mx + TILE - 1) // TILE

    # schedule: edge stream order = (range, q, t); offsets in chunks
    nrange = (cfg.NT + T_RANGE - 1) // T_RANGE
    chunk_off = np.zeros((cfg.NT, NQ), np.int64)
    units = []   # (r, q, edge_off, n_edges)
    off = 0
    for r in range(nrange):
        tiles = list(range(r * T_RANGE, min((r + 1) * T_RANGE, cfg.NT)))
        for q in range(NQ):
            u0 = off
            for t in tiles:
                chunk_off[t, q] = off
                off += C[t, q]
            units.append((r, q, u0 * TILE, (off - u0) * TILE))
    totch = off
    tot = totch * TILE

    # source q-tables (global x, identical on all cores): 256B bf16 rows
    xbf = x.astype(ml_dtypes.bfloat16)
    trows = 4 * cfg.S
    xq_tab = np.zeros((NQ, trows, cfg.D_IN), ml_dtypes.bfloat16)
    for q in range(NQ):
        xq_tab[q] = np.concatenate(
            [xbf[bb * cfg.BLOCK + q * cfg.S:bb * cfg.BLOCK + q * cfg.S + cfg.S]
             for bb in range(4)], axis=0)          # [4*S, D_IN]
    xq_tab = np.ascontiguousarray(xq_tab)

    in_maps = []
    for k in range(N_CORES):
        g, b = k % 2, k // 2
        buckets = core_buckets[k]
        inv_blk = streams[g][2][b * cfg.BLOCK:(b + 1) * cfg.BLOCK]
        col_l = np.zeros(tot, np.int16)
        dest_l = np.full(tot, -1.0, np.float32)
        for t in range(cfg.NT):
            for q in range(NQ):
                cl, slot = buckets[t][q]
                o = chunk_off[t, q] * TILE
                n = len(cl)
                col_l[o:o + n] = cl.astype(np.int16)
                dest_l[o:o + n] = slot.astype(np.float32)
        eidx = np.tile(col_l.reshape(tot // 16, 16).T, (8, 1))  # [128, tot/16]
        edest = np.ascontiguousarray(
            dest_l.reshape(totch, TILE).T.astype(ml_dtypes.bfloat16))
        invC = np.zeros((128, cfg.NT), np.float32)              # [slot, tile]
        for t in range(cfg.NT):
            w = min(TILE, cfg.BLOCK - t * TILE)
            invC[:w, t] = inv_blk[t * TILE:t * TILE + w]

        xT = xbf[b * cfg.BLOCK:(b + 1) * cfg.BLOCK].T.copy()    # [D_IN, BLOCK]

        pre = "s" if g == 0 else "a"
        sc = np.float32(w_blend if g == 0 else 1.0 - w_blend)
        w0s = np.asarray(inputs[f"{pre}0_ws"], np.float32).astype(ml_dtypes.bfloat16)
        w0n = np.asarray(inputs[f"{pre}0_wn"], np.float32).astype(ml_dtypes.bfloat16)
        w1s = (np.asarray(inputs[f"{pre}1_ws"], np.float32) * sc).astype(ml_dtypes.bfloat16)
        w1n = (np.asarray(inputs[f"{pre}1_wn"], np.float32) * sc).astype(ml_dtypes.bfloat16)
        b0 = (np.asarray(inputs[f"{pre}0_bs"], np.float32)
              + np.asarray(inputs[f"{pre}0_bn"], np.float32))
        b1 = (np.asarray(inputs[f"{pre}1_bs"], np.float32)
              + np.asarray(inputs[f"{pre}1_bn"], np.float32)) * sc

        in_maps.append({
            "xp": xq_tab, "xT": xT,
            "eidx": eidx, "edest": edest, "invC": invC,
            "w0s": w0s, "w0n": w0n,
            "w1s0": w1s[:128].copy(), "w1s1": w1s[128:].copy(),
            "w1n0": w1n[:128].copy(), "w1n1": w1n[128:].copy(),
            "b0": b0.reshape(2, 128).T.copy(),   # [128, 2]
            "b1": b1.reshape(2, 128).T.copy(),
        })

    umax = 0
    for r in range(nrange):
        tiles = list(range(r * T_RANGE, min((r + 1) * T_RANGE, cfg.NT)))
        for q in range(NQ):
            umax = max(umax, int(sum(C[t, q] for t in tiles)))
    sched = dict(C=C, chunk_off=chunk_off, units=units, totch=totch, tot=tot,
                 nrange=nrange, maxch=int(C.max()), umax=umax)
    return in_maps, sched


# ---------------------------------------------------------------- program

def build_program(cfg, sched):
    NOCOLL = os.environ.get("GNN_NOCOLL") == "1"
    REPEAT = int(os.environ.get("GNN_REPEAT", "1"))
    NQUEUES = int(os.environ.get("GNN_QUEUES", "4"))
    SHARED_CC = os.environ.get("GNN_SHARED_CC") == "1"
    SKIPAGG = os.environ.get("GNN_SKIPAGG") == "1"   # timing ablation only
    GATHERONLY = os.environ.get("GNN_GATHERONLY") == "1"  # timing ablation only
    NOGATHER = os.environ.get("GNN_NOGATHER") == "1"      # timing ablation only
    G0BUFS = int(os.environ.get("GNN_G0BUFS", "6"))
    G1BUFS = int(os.environ.get("GNN_G1BUFS", "2"))
    C, chunk_off = sched["C"], sched["chunk_off"]
    totch, tot = sched["totch"], sched["tot"]
    MAXCH = sched["maxch"]
    DH = cfg.D_HID
    S = cfg.S

    nc = bacc.Bacc("TRN2", target_bir_lowering=False, debug=False,
                   num_devices=1 if NOCOLL else N_CORES,
                   num_swdge_queues=NQUEUES,
                   dynamic_dma_scratch_size=int(
                       os.environ.get("GNN_SCRATCH", "8192")))

    xp_d = nc.dram_tensor("xp", [NQ, 4 * cfg.S, cfg.D_IN], BF16,
                          kind="ExternalInput")
    xT_d = nc.dram_tensor("xT", [cfg.D_IN, cfg.BLOCK], BF16, kind="ExternalInput")
    eidx_d = nc.dram_tensor("eidx", [128, tot // 16], I16, kind="ExternalInput")
    edest_d = nc.dram_tensor("edest", [128, totch], BF16, kind="ExternalInput")
    invC_d = nc.dram_tensor("invC", [128, cfg.NT], F32, kind="ExternalInput")
    w0s_d = nc.dram_tensor("w0s", [cfg.D_IN, DH], BF16, kind="ExternalInput")
    w0n_d = nc.dram_tensor("w0n", [cfg.D_IN, DH], BF16, kind="ExternalInput")
    w1_d = {(nm, kk): nc.dram_tensor(f"w1{nm}{kk}", [128, cfg.D_OUT], BF16,
                                     kind="ExternalInput")
            for nm in ("s", "n") for kk in (0, 1)}
    b0_d = nc.dram_tensor("b0", [128, 2], F32, kind="ExternalInput")
    b1_d = nc.dram_tensor("b1", [128, 2], F32, kind="ExternalInput")
    yT_d = nc.dram_tensor("yT", [cfg.D_OUT, cfg.BLOCK], F32, kind="ExternalOutput")
    h0full_in = ([nc.dram_tensor(f"h0full{q}", [4 * cfg.S, DH], BF16,
                                 kind="ExternalInput") for q in range(NQ)]
                 if NOCOLL else None)

    AG_GROUPS = [[0, 2, 4, 6], [1, 3, 5, 7]]
    AR_GROUPS = [[0, 1], [2, 3], [4, 5], [6, 7]]

    with tile.TileContext(nc) as tc:
        with (
            tc.tile_pool(name="const", bufs=1) as cp,
            tc.tile_pool(name="p", bufs=3) as pp,
            tc.tile_pool(name="stage", bufs=2) as sp,
            tc.tile_pool(name="dram", bufs=1, space="DRAM") as dram,
            tc.tile_pool(name="h0p", bufs=1) as h0p,
        ):
            # ---- constants
            edest_t = cp.tile([128, totch], BF16)
            invC_t = cp.tile([128, cfg.NT], F32)
            eidx_t = cp.tile([128, tot // 16], I16)
            w0s_t = cp.tile([cfg.D_IN, DH], BF16)
            w0n_t = cp.tile([cfg.D_IN, DH], BF16)
            w1_t = {k: cp.tile([128, cfg.D_OUT], BF16, name=f"w1{k[0]}{k[1]}",
                               tag=f"w1{k[0]}{k[1]}") for k in w1_d}
            b0_t = cp.tile([128, 2], F32)
            b1_t = cp.tile([128, 2], F32)
            iota_i = cp.tile([128, TILE], I16)
            iota_bf = cp.tile([128, TILE], BF16)
            ident = cp.tile([128, TILE], BF16)
            pidx_i = cp.tile([128, 1], I16)
            pidx_f = cp.tile([128, 1], F32)

            nc.sync.dma_start(edest_t[:], edest_d[:])
            nc.sync.dma_start(invC_t[:], invC_d[:])
            nc.sync.dma_start(eidx_t[:], eidx_d[:])
            nc.sync.dma_start(w0s_t[:], w0s_d[:])
            nc.sync.dma_start(w0n_t[:], w0n_d[:])
            for k in w1_d:
                nc.sync.dma_start(w1_t[k][:], w1_d[k][:])
            nc.sync.dma_start(b0_t[:], b0_d[:])
            nc.sync.dma_start(b1_t[:], b1_d[:])
            nc.gpsimd.iota(iota_i[:], pattern=[[1, TILE]], base=0,
                           channel_multiplier=0)
            nc.vector.tensor_copy(iota_bf[:], iota_i[:])
            nc.gpsimd.iota(pidx_i[:], pattern=[[1, 1]], base=0,
                           channel_multiplier=1)
            nc.vector.tensor_copy(pidx_f[:], pidx_i[:])
            nc.vector.tensor_scalar(ident[:], iota_bf[:], pidx_f[:], None,
                                    mybir.AluOpType.is_equal)

            # ---- DRAM bounces
            h0rm_q = [dram.tile([S, DH], BF16, name=f"h0rm{q}", tag=f"h0rm{q}")
                      for q in range(NQ)]
            cc_space = "Shared" if SHARED_CC else "Local"
            h0full_q = (h0full_in if NOCOLL else
                        [dram.tile([4 * cfg.S, DH], BF16, name=f"h0f{q}",
                                   tag=f"h0f{q}", addr_space=cc_space)
                         for q in range(NQ)])
            yar_in = [dram.tile([cfg.D_OUT, S], BF16, name=f"yi{c}", tag=f"yi{c}")
                      for c in range(NQ)]
            yar_out = [dram.tile([cfg.D_OUT, S], BF16, name=f"yo{c}",
                                 tag=f"yo{c}", addr_space=cc_space)
                       for c in range(NQ)]

            h0T = [h0p.tile([128, cfg.BLOCK], BF16, name=f"h0T{m}",
                            tag=f"h0T{m}") for m in range(2)]

            PBMAX = sched["umax"]

            def gen_pb(gc0, nch):
                """Pure one-hot matrices for nch chunks starting at gc0, ONE
                batched DVE is_equal over broadcast APs (1/deg is applied
                later as a per-partition scale on the dest-major PSUM).
                pb[:, c, j] = (iota[j] == edest[:, gc0+c])"""
                pb = pp.tile([128, PBMAX, TILE], BF16, tag="p")
                nc.vector.tensor_tensor(
                    pb[:, :nch, :],
                    iota_bf.unsqueeze(1).broadcast_to([128, nch, TILE]),
                    edest_t[:, gc0:gc0 + nch].unsqueeze(2).broadcast_to(
                        [128, nch, TILE]),
                    mybir.AluOpType.is_equal)
                return pb

            qctr = [0]

            # slice each unit's gather into ring-sized pieces spread
            # round-robin over the SWDGE queues: parallelizes one unit's
            # transfer across queues and keeps every piece under the
            # per-queue descriptor ring (scratch/16 descs).
            GCH = int(os.environ.get("GNN_GCH", "12"))  # chunks per piece

            def gather_unit(gp, idx_view, ne, src_ap, tag, elem):
                gt = gp.tile([128, max(ne // TILE, 1), elem], BF16, tag=tag)
                nch = ne // TILE
                for o in range(0, nch, GCH):
                    k = min(GCH, nch - o)
                    nc.gpsimd.dma_gather(
                        gt[:, o:o + k, :], src_ap,
                        idx_view[:, o * TILE // 16:(o + k) * TILE // 16],
                        num_idxs=k * TILE, num_idxs_reg=k * TILE,
                        elem_size=elem, single_packet=False,
                        queue_num=qctr[0] % NQUEUES)
                    qctr[0] += 1
                return gt

            for _rep in range(REPEAT):
                # ================= L0 =================
                with nc.named_scope(f"L0_{_rep}"), \
                     tc.tile_pool(name=f"l0big{_rep}", bufs=1) as l0big, \
                     tc.tile_pool(name=f"g0p{_rep}", bufs=G0BUFS) as g0p, \
                     tc.tile_pool(name=f"ps0{_rep}", bufs=2, space="PSUM") as psp:
                    neiT = l0big.tile([128, cfg.BLOCK], BF16, tag="neiT")
                    if SKIPAGG or GATHERONLY:
                        nc.vector.memset(neiT[:], 0.0)
                    for r in range(sched["nrange"] if not SKIPAGG else 0):
                        tiles = list(range(r * T_RANGE,
                                           min((r + 1) * T_RANGE, cfg.NT)))
                        e0r = chunk_off[tiles[0], 0] * TILE
                        ner = sum(int(C[t, q]) for t in tiles
                                  for q in range(NQ)) * TILE
                        if ner == 0:
                            for t in tiles:
                                w = TILE if t < cfg.NT - 1 else cfg.LAST_W
                                nc.vector.memset(
                                    neiT[:, t * TILE:t * TILE + w], 0.0)
                            continue
                        gts = {}
                        for q in range(NQ):
                            e0 = chunk_off[tiles[0], q] * TILE
                            ne = sum(int(C[t, q]) for t in tiles) * TILE
                            if ne:
                                gts[q] = (gather_unit(
                                    g0p, eidx_t[:, e0 // 16:(e0 + ne) // 16],
                                    ne, xp_d[q, :, :], "g0", cfg.D_IN),
                                    chunk_off[tiles[0], q],
                                    gen_pb(chunk_off[tiles[0], q] * TILE
                                           // TILE, ne // TILE))
                        for t in (tiles if not GATHERONLY else []):
                            w = TILE if t < cfg.NT - 1 else cfg.LAST_W
                            nch = int(sum(C[t, q] for q in range(NQ)))
                            if nch == 0:
                                nc.vector.memset(neiT[:, t * TILE:t * TILE + w],
                                                 0.0)
                                continue
                            ps = psp.tile([128, cfg.D_IN], F32,
                                          name=f"nei0_{t}", tag="nei0", bufs=3)
                            done = 0
                            for q in range(NQ):
                                nq_ch = int(C[t, q])
                                if not nq_ch:
                                    continue
                                gt, base, pb = gts[q]
                                gc0 = int(chunk_off[t, q])
                                for c in range(nq_ch):
                                    lc = gc0 + c - int(base)
                                    nc.tensor.matmul(
                                        ps[:], pb[:, lc, :], gt[:, lc, :],
                                        start=(done == 0),
                                        stop=(done == nch - 1))
                                    done += 1
                            # dest-major [d, f]: fold 1/deg (per-partition)
                            tmp = sp.tile([128, cfg.D_IN], BF16, tag="agg0")
                            nc.scalar.activation(
                                tmp[:w, :], ps[:w, :],
                                mybir.ActivationFunctionType.Copy,
                                scale=invC_t[:w, t:t + 1])
                            pst = psp.tile([128, TILE], BF16,
                                           name=f"tr0_{t}", tag="tr0", bufs=2)
                            nc.tensor.transpose(pst[:, :w], tmp[:w, :],
                                                ident[:w, :w])
                            nc.vector.tensor_copy(
                                neiT[:, t * TILE:t * TILE + w], pst[:, :w])

                    # dense L0 (xT streamed per chunk)
                    for j in range(cfg.NJ):
                        sl = slice(j * cfg.DENSE_N, (j + 1) * cfg.DENSE_N)
                        xT_j = sp.tile([cfg.D_IN, cfg.DENSE_N], BF16, tag="xTj")
                        nc.sync.dma_start(xT_j[:], xT_d[:, sl])
                        for m in range(2):
                            ps = psp.tile([128, cfg.DENSE_N], F32,
                                          name=f"d0_{m}_{j}", tag="d")
                            nc.tensor.matmul(ps[:], w0s_t[:, m * 128:(m + 1) * 128],
                                             xT_j[:], start=True, stop=False)
                            nc.tensor.matmul(ps[:], w0n_t[:, m * 128:(m + 1) * 128],
                                             neiT[:, sl], start=False, stop=True)
                            nc.scalar.activation(h0T[m][:, sl], ps[:],
                                                 mybir.ActivationFunctionType.Relu,
                                                 bias=b0_t[:, m:m + 1])

                # ======== row-major h0 (+ chunked AllGather) ========
                with nc.named_scope(f"H0X_{_rep}"), \
                     tc.tile_pool(name=f"pstr{_rep}", bufs=4, space="PSUM") as pstr:
                    for t in range(cfg.NT):
                        w = TILE if t < cfg.NT - 1 else cfg.LAST_W
                        rm = sp.tile([128, DH], BF16, tag="rm")
                        for m in range(2):
                            pst = pstr.tile([128, TILE], BF16, name=f"tr_{t}_{m}",
                                            tag="tr")
                            nc.tensor.transpose(pst[:w, :],
                                                h0T[m][:, t * TILE:t * TILE + w],
                                                ident[:])
                            if m == 0:
                                nc.vector.tensor_copy(rm[:w, :128], pst[:w, :])
                            else:
                                nc.scalar.activation(
                                    rm[:w, 128:], pst[:w, :],
                                    mybir.ActivationFunctionType.Copy)
                        # split rows across q chunk boundaries
                        r0 = t * TILE
                        for q in range(r0 // S, (r0 + w - 1) // S + 1):
                            a = max(r0, q * S)
                            bnd = min(r0 + w, (q + 1) * S)
                            nc.sync.dma_start(h0rm_q[q][a - q * S:bnd - q * S, :],
                                              rm[a - r0:bnd - r0, :])
                    if not NOCOLL:
                        for q in range(NQ):
                            nc.gpsimd.collective_compute(
                                "AllGather", mybir.AluOpType.bypass,
                                ins=[h0rm_q[q].opt()], outs=[h0full_q[q].opt()],
                                replica_groups=AG_GROUPS)

                # ================= L1 =================
                with nc.named_scope(f"L1_{_rep}"), \
                     tc.tile_pool(name=f"l1big{_rep}", bufs=1) as l1big, \
                     tc.tile_pool(name=f"g1p{_rep}", bufs=G1BUFS) as g1p, \
                     tc.tile_pool(name=f"ps1{_rep}", bufs=2, space="PSUM") as psp1:
                    nei1T = [l1big.tile([128, cfg.BLOCK], BF16, name=f"nei1T{m}",
                                        tag=f"nei1T{m}") for m in range(2)]
                    if SKIPAGG or GATHERONLY:
                        for m in range(2):
                            nc.vector.memset(nei1T[m][:], 0.0)
                    for r in range(sched["nrange"] if not SKIPAGG else 0):
                        tiles = list(range(r * T_RANGE,
                                           min((r + 1) * T_RANGE, cfg.NT)))
                        e0r = chunk_off[tiles[0], 0] * TILE
                        ner = sum(int(C[t, q]) for t in tiles
                                  for q in range(NQ)) * TILE
                        if ner == 0:
                            for t in tiles:
                                w = TILE if t < cfg.NT - 1 else cfg.LAST_W
                                for m in range(2):
                                    nc.vector.memset(
                                        nei1T[m][:, t * TILE:t * TILE + w], 0.0)
                            continue
                        gts = {}
                        for q in range(NQ):
                            e0 = chunk_off[tiles[0], q] * TILE
                            ne = sum(int(C[t, q]) for t in tiles) * TILE
                            if ne:
                                gts[q] = (gather_unit(
                                    g1p, eidx_t[:, e0 // 16:(e0 + ne) // 16],
                                    ne, h0full_q[q][:, :], "g1", 2 * cfg.D_IN),
                                    chunk_off[tiles[0], q],
                                    gen_pb(int(chunk_off[tiles[0], q]),
                                           ne // TILE))
                        for t in (tiles if not GATHERONLY else []):
                            w = TILE if t < cfg.NT - 1 else cfg.LAST_W
                            nch = int(sum(C[t, q] for q in range(NQ)))
                            if nch == 0:
                                for m in range(2):
                                    nc.vector.memset(
                                        nei1T[m][:, t * TILE:t * TILE + w], 0.0)
                                continue
                            ps = psp1.tile([128, 2 * cfg.D_IN], F32,
                                           name=f"n1_{t}", tag="n1", bufs=3)
                            done = 0
                            for q in range(NQ):
                                nq_ch = int(C[t, q])
                                if not nq_ch:
                                    continue
                                gt, base, pb = gts[q]
                                gc0 = int(chunk_off[t, q])
                                for c in range(nq_ch):
                                    lc = gc0 + c - int(base)
                                    nc.tensor.matmul(
                                        ps[:], pb[:, lc, :], gt[:, lc, :],
                                        start=(done == 0),
                                        stop=(done == nch - 1))
                                    done += 1
                            # dest-major [d, 256f]: fold 1/deg per-partition
                            tmp = sp.tile([128, 2 * cfg.D_IN], BF16, tag="agg1")
                            nc.scalar.activation(
                                tmp[:w, :], ps[:w, :],
                                mybir.ActivationFunctionType.Copy,
                                scale=invC_t[:w, t:t + 1])
                            for m in range(2):
                                pst = psp1.tile([128, TILE], BF16,
                                                name=f"tr1_{t}_{m}", tag="tr1",
                                                bufs=3)
                                nc.tensor.transpose(
                                    pst[:, :w],
                                    tmp[:w, m * 128:(m + 1) * 128],
                                    ident[:w, :w])
                                if m == 0:
                                    nc.vector.tensor_copy(
                                        nei1T[m][:, t * TILE:t * TILE + w],
                                        pst[:, :w])
                                else:
                                    nc.scalar.activation(
                                        nei1T[m][:, t * TILE:t * TILE + w],
                                        pst[:, :w],
                                        mybir.ActivationFunctionType.Copy)

                    # dense L1 -> yar chunks (bf16)
                    for j in range(cfg.NJ):
                        sl = slice(j * cfg.DENSE_N, (j + 1) * cfg.DENSE_N)
                        for m in range(2):
                            ps = psp1.tile([128, cfg.DENSE_N], F32,
                                           name=f"d1_{m}_{j}", tag="d")
                            nc.tensor.matmul(
                                ps[:], w1_t[("s", 0)][:, m * 128:(m + 1) * 128],
                                h0T[0][:, sl], start=True, stop=False)
                            nc.tensor.matmul(
                                ps[:], w1_t[("s", 1)][:, m * 128:(m + 1) * 128],
                                h0T[1][:, sl], start=False, stop=False)
                            nc.tensor.matmul(
                                ps[:], w1_t[("n", 0)][:, m * 128:(m + 1) * 128],
                                nei1T[0][:, sl], start=False, stop=False)
                            nc.tensor.matmul(
                                ps[:], w1_t[("n", 1)][:, m * 128:(m + 1) * 128],
                                nei1T[1][:, sl], start=False, stop=True)
                            st = sp.tile([128, cfg.DENSE_N], BF16, tag="h1")
                            nc.scalar.activation(st[:], ps[:],
                                                 mybir.ActivationFunctionType.Relu,
                                                 bias=b1_t[:, m:m + 1])
                            # split cols across AR chunk boundaries
                            c0 = j * cfg.DENSE_N
                            for c in range(c0 // S,
                                           (c0 + cfg.DENSE_N - 1) // S + 1):
                                a = max(c0, c * S)
                                bnd = min(c0 + cfg.DENSE_N, (c + 1) * S)
                                nc.sync.dma_start(
                                    yar_in[c][m * 128:(m + 1) * 128,
                                              a - c * S:bnd - c * S],
                                    st[:, a - c0:bnd - c0])

                # ======== chunked AllReduce + output ========
                with nc.named_scope(f"AR_{_rep}"):
                    for c in range(NQ):
                        if NOCOLL:
                            src = yar_in[c]
                        else:
                            nc.gpsimd.collective_compute(
                                "AllReduce", mybir.AluOpType.add,
                                ins=[yar_in[c].opt()], outs=[yar_out[c].opt()],
                                replica_groups=AR_GROUPS)
                            src = yar_out[c]
                        CC = 625
                        for m in range(2):
                            for cc in range(S // CC):
                                csl = slice(cc * CC, (cc + 1) * CC)
                                lt = sp.tile([128, CC], BF16, tag="lt")
                                nc.sync.dma_start(
                                    lt[:], src[m * 128:(m + 1) * 128, csl])
                                ft = sp.tile([128, CC], F32, tag="ft")
                                nc.scalar.activation(
                                    ft[:], lt[:],
                                    mybir.ActivationFunctionType.Copy)
                                nc.sync.dma_start(
                                    yT_d[m * 128:(m + 1) * 128,
                                         c * S + cc * CC:c * S + (cc + 1) * CC],
                                    ft[:])

    nc.compile()
    return nc


# ---------------------------------------------------------------- entry

_CACHE = {}


def _build(inputs, cfg):
    in_maps, sched = preprocess(inputs, cfg)
    key = (cfg.N, cfg.E, sched["tot"])
    if key not in _CACHE:
        _CACHE[key] = build_program(cfg, sched)
    return _CACHE[key], in_maps


def run_config(inputs, cfg):
    nc, in_maps = _build(inputs, cfg)
    from concourse import bass2jax
    results = bass2jax.run_bass_via_pjrt(nc, in_maps, n_cores=N_CORES)
    blocks = [results[2 * b]["yT"].T for b in range(4)]
    return np.ascontiguousarray(np.concatenate(blocks, axis=0), dtype=np.float32)


def kernel(**inputs):
    return run_config(inputs, FULL)
